# revision 45
# baseline (speedup 1.0000x reference)
"""Trainium2 Bass kernel for MultiHeadAttention + residual + BatchNorm.

Model (reference):
  q = query @ Wq.T ; k = key @ Wk.T ; v = key @ Wv.T    (per-head split)
  score = q k^T / sqrt(D), causal mask, softmax over keys
  res   = (attn @ v) + query
  out   = batchnorm(res over all (N*L) rows, per feature) * gamma + beta

Sharding over 8 cores: FEATURE sharding. Core c owns heads {2c, 2c+1}
(features [128c, 128c+128)) for ALL batches. BatchNorm statistics are
then core-local (sums over all N*L rows of the core's own features), so
no collective is needed at all.

All matmul operands are bf16 (PSUM accumulation stays fp32); the
residual add, batch-norm statistics and outputs are fp32.

Per 512-row query chunk ic and batch n the attention inner loop walks
key blocks jc (128 keys each, causal): PE computes both heads' scores
into one PSUM tile [128 j, 1024 (h,i)], ScalarE applies a single merged
exp -> bf16, DVE masks the diagonal block with a triangular multiply,
and PE accumulates the *flipped* attention-V product out[i, p] with a
ones-column appended to V so softmax denominators fall out of the same
matmuls. Projections for chunk ic+1 are emitted as small PE micro-ops
paced between attention iterations so the scalar engine never starves.
"""

import math
import sys

sys.path.insert(0, "/opt/trn_rl_repo")

import numpy as np
import ml_dtypes

import concourse.bass as bass
import concourse.mybir as mybir
from concourse import bacc
import concourse.tile as tile

F32 = mybir.dt.float32
F32R = mybir.dt.float32r
BF16 = mybir.dt.bfloat16
BF16_NP = ml_dtypes.bfloat16

N = 4
L = 2048
D = 1024
H = 16
P = 64
NCORES = 8
FC = D // NCORES       # features per core = 128
H2 = 2                 # heads per core
EPS = 1e-5
SCALE = 1.0 / math.sqrt(D)
NL = N * L             # 8192 rows in the global batch norm

# software-pipeline stage lags (consumers trail producers so engine wait
# queues never backpressure the sequencers)
EXP_LAG = 0
MASK_LAG = 0
AV_LAG = 0
DEBUG_NOLAG = True

_cached = {}


def r(ap):
    return ap.bitcast(F32R)


def build_program(l=L):
    """Build the SPMD Bass program (identical on all 8 cores)."""
    nc = bacc.Bacc("TRN2", target_bir_lowering=False, debug=False,
                   num_devices=NCORES)

    ic_n = l // 512        # 512-row query chunks
    nlc = N * l            # rows per core (all batches)

    xq_nd = nc.dram_tensor("xq_nd", [D, nlc], BF16, kind="ExternalInput").ap()
    xk_nd = nc.dram_tensor("xk_nd", [D, nlc], BF16, kind="ExternalInput").ap()
    wqt = nc.dram_tensor("wqt", [D, FC], BF16, kind="ExternalInput").ap()
    wkt = nc.dram_tensor("wkt", [D, FC], BF16, kind="ExternalInput").ap()
    wvt = nc.dram_tensor("wvt", [D, FC], BF16, kind="ExternalInput").ap()
    q_res = nc.dram_tensor("q_res", [nlc, FC], BF16, kind="ExternalInput").ap()
    gamma = nc.dram_tensor("gamma", [1, FC], F32, kind="ExternalInput").ap()
    beta = nc.dram_tensor("beta", [1, FC], F32, kind="ExternalInput").ap()
    out_s = nc.dram_tensor("out_s", [nlc, FC], BF16,
                           kind="ExternalOutput").ap()

    jblocks = l // 128     # 16 key blocks per batch
    nls = nlc // 128       # 64 ls blocks of res

    from contextlib import ExitStack
    with ExitStack() as stack:
        tc = stack.enter_context(tile.TileContext(nc))
        pool = {}
        for nm, bufs, space in (
                ("consts", 1, None), ("persist", 1, None), ("wt", 1, None),
                ("qtp", 2, None), ("xq", 2, None), ("xk", 2, None),
                ("at2", 3, None), ("qin", 2, None), ("sq", 2, None),
                ("outp", 3, None), ("bnp", 1, None), ("small", 6, None),
                ("st2", 2, "PSUM"), ("av", 1, "PSUM"), ("pj", 1, "PSUM"),
                ("stat", 1, "PSUM")):
            kw = {"name": nm, "bufs": bufs}
            if space:
                kw["space"] = space
            pool[nm] = stack.enter_context(tc.tile_pool(**kw))
        consts, persist, wtp = pool["consts"], pool["persist"], pool["wt"]
        qtp, xqp, xkp = pool["qtp"], pool["xq"], pool["xk"]
        at2p, qinp, sqp = pool["at2"], pool["qin"], pool["sq"]
        outp, bnp, smallp = pool["outp"], pool["bnp"], pool["small"]
        st2p, avp, pjp, statp = (pool["st2"], pool["av"], pool["pj"],
                                 pool["stat"])

        # ---------------- first activation chunk DMAs (critical path) ----
        def x_src(dram, n, ic):
            return bass.AP(
                tensor=dram.tensor,
                offset=dram.offset + n * l + ic * 512,
                ap=[[nlc, 128], [128 * nlc, 8], [1, 512]])

        def load_x(pool, dram, n, ic):
            t = pool.tile([128, 8 * 512], BF16, tag="x", name="xt")
            nc.sync.dma_start(
                t.rearrange("p (dc x) -> p dc x", dc=8), x_src(dram, n, ic))
            return t

        # first q-projection chain needs wq then xq: emit those two DMAs
        # first so PE can start as early as possible
        wts = {}

        def load_w(wname, wdram):
            t = wtp.tile([128, 8 * FC], BF16, tag=wname, name=wname)
            nc.sync.dma_start(
                t.rearrange("p (dc f) -> p dc f", dc=8),
                bass.AP(tensor=wdram.tensor, offset=wdram.offset,
                        ap=[[FC, 128], [128 * FC, 8], [1, FC]]))
            wts[wname] = t

        load_w("wq", wqt)
        xq_t = load_x(xqp, xq_nd, 0, 0)
        load_w("wk", wkt)
        xk_t = load_x(xkp, xk_nd, 0, 0)
        load_w("wv", wvt)

        # ---------------- constants -------------------------------------
        ones_col = consts.tile([128, 1], BF16)
        nc.vector.memset(ones_col, 1.0)
        eps_sb = consts.tile([128, 1], F32)
        nc.vector.memset(eps_sb, EPS)
        gamma_sb = consts.tile([1, FC], F32)
        nc.sync.dma_start(gamma_sb, gamma)
        beta_sb = consts.tile([1, FC], F32)
        nc.sync.dma_start(beta_sb, beta)
        # lower-triangular (j <= i) mask in [j-part, i-free] layout
        tm_f = consts.tile([128, 128], F32)
        nc.vector.memset(tm_f, 1.0)
        nc.gpsimd.affine_select(
            out=tm_f, in_=tm_f,
            compare_op=mybir.AluOpType.is_ge, fill=0.0, base=0,
            pattern=[[1, 128]], channel_multiplier=-1)
        trimask = consts.tile([128, 128], BF16)
        nc.vector.tensor_copy(trimask, tm_f)
        # zero operands for the PSUM-bank-clearing matmuls (PSUM
        # start_tensor_calc marks a whole 2KB zero-region, so banks shared
        # by several accumulation groups must be cleared by one explicit
        # full-tile start matmul instead of per-group start flags)
        zlhs = consts.tile([128, 128], BF16)
        nc.vector.memset(zlhs, 0.0)
        zrhs = consts.tile([128, 260], BF16)
        nc.vector.memset(zrhs, 0.0)

        # ---------------- persistent SBUF -------------------------------
        # kt_sb: [feature(h*64+p), (n, j)] bf16
        kt_sb = persist.tile([128, N * l], BF16, tag="kt")
        # v_sb: [j-in-block, (n, jc, h, 65)] bf16; col 64 of each 65-group
        # is the baked ones column (softmax denominator trick)
        v_sb = persist.tile([128, N * jblocks * H2 * 65], BF16, tag="v")
        v3 = v_sb.rearrange("p (g x) -> p g x", x=65)
        nc.gpsimd.memset(v3[:, :, 64:65], 1.0)
        # res_sb: [l-in-block, (ls, f)] bf16, ls = n*16 + ic*4 + S
        res_sb = persist.tile([128, nls * FC], BF16, tag="res")

        # partition 0 row: sums; partition 64 row: sums of squares
        # (matmul outputs must start at partition 0, 32, or 64)
        stat = statp.tile([65, 512], F32, tag="stat")

        # ------------------------------------------------------------------
        # projection task machinery (filler micro-ops paced into B loops)
        # ------------------------------------------------------------------
        # rotating PSUM allocators: the interleaved fillers use the single
        # pj bank; the up-front A(0) block also rotates through the two
        # (then idle) score banks so chains overlap their copy-out
        def alloc_pj():
            return pjp.tile([128, 512], F32, tag="pj", name="pj")

        def alloc_st2_slot():
            return st2p.tile([128, 1024], F32, tag="st2", name="st2")[:, 0:512]

        rot = {"i": 0}

        def alloc_rotating():
            rot["i"] += 1
            return alloc_pj() if rot["i"] % 3 == 0 else alloc_st2_slot()

        chain_alloc = {"fn": alloc_pj}

        def qk_chain(side, n, ic, qt_tile, get_xt):
            """Emit one q/k projection chain as a list of micro closures."""
            w_use = wts["wq"] if side == "q" else wts["wk"]
            pj = {}
            alloc_fn = chain_alloc["fn"]

            def alloc():
                pj["t"] = alloc_fn()

            def mm(dc0):
                xt = get_xt()
                for dc in (dc0, dc0 + 1):
                    nc.tensor.matmul(
                        pj["t"],
                        wts_slice(w_use, dc),
                        xt[:, dc * 512:(dc + 1) * 512],
                        start=(dc == 0), stop=(dc == 7))

            def copy():
                if side == "q":
                    nc.vector.tensor_copy(qt_tile[:, n * 512:(n + 1) * 512],
                                          pj["t"])
                else:
                    nc.vector.tensor_copy(
                        kt_sb[:, n * l + ic * 512:n * l + ic * 512 + 512],
                        pj["t"])

            ops = [alloc]
            for dc0 in range(0, 8, 2):
                ops.append(lambda d=dc0: mm(d))
            ops.append(copy)
            return ops

        def wts_slice(w, dc):
            return w[:, dc * FC:(dc + 1) * FC]

        def v_chain(n, jsub, ic, get_xt):
            pj = {}
            alloc_fn = chain_alloc["fn"]

            def alloc():
                pj["t"] = alloc_fn()

            def mm(dc0):
                xt = get_xt()
                for dc in (dc0, dc0 + 1):
                    nc.tensor.matmul(
                        pj["t"][:, 0:128],
                        xt[:, dc * 512 + jsub * 128:dc * 512 + jsub * 128 + 128],
                        wts_slice(wts["wv"], dc),
                        start=(dc == 0), stop=(dc == 7))

            def copy():
                jc = ic * 4 + jsub
                base = (n * jblocks + jc) * H2 * 65
                dst = v_sb[:, base:base + 130].rearrange(
                    "p (h x) -> p h x", h=2)[:, :, 0:64]
                src = pj["t"][:, 0:128].rearrange("p (h x) -> p h x", h=2)
                nc.vector.tensor_copy(dst, src)

            ops = [alloc]
            for dc0 in range(0, 8, 2):
                ops.append(lambda d=dc0: mm(d))
            ops.append(copy)
            return ops

        def build_chunk_groups(ic, qt_tile, first_x, rotate_first=False):
            """Return per-batch lists of micro-op closures for A(ic)."""
            groups = []
            xq_cur = {0: first_x[0]}
            xk_cur = {0: first_x[1]}
            for n in range(N):
                chain_alloc["fn"] = (alloc_rotating if rotate_first and n == 0
                                     else alloc_pj)
                ops = []
                get_xq = lambda nn=n: xq_cur[nn]
                get_xk = lambda nn=n: xk_cur[nn]
                # prefetch next batch's activations
                if n + 1 < N:
                    def pre(nn=n + 1):
                        xq_cur[nn] = load_x(xqp, xq_nd, nn, ic)
                        xk_cur[nn] = load_x(xkp, xk_nd, nn, ic)
                    ops.append(pre)
                ops += qk_chain("q", n, ic, qt_tile, get_xq)
                ops += qk_chain("k", n, ic, None, get_xk)
                for jsub in range(4):
                    ops += v_chain(n, jsub, ic, get_xk)
                groups.append(ops)
            return groups

        # ------------------------------------------------------------------
        # A(0): only batch 0's projections run up front; batches 1-3 are
        # deadline-paced into B(0)'s iterations (batch n is needed at
        # B(0) iteration 4n) so the scalar engine starts exp'ing early.
        # ------------------------------------------------------------------
        qt_next = qtp.tile([128, N * 512], BF16, tag="qt", name="qt")
        groups0 = build_chunk_groups(0, qt_next, (xq_t, xk_t),
                                     rotate_first=True)
        for op in groups0[0]:
            op()

        # ------------------------------------------------------------------
        # main loop: one software-pipelined stream over (ic, n, jc).
        # Stage schedule at step t: scores(t), exp(t-1), mask(t-2), AV(t-3)
        # so every instruction's inputs are ready when the engine decodes
        # it (the 4-deep per-engine wait queues otherwise backpressure the
        # sequencers). A(ic+1) projection micro-ops are paced in as filler.
        # ------------------------------------------------------------------
        specs = []
        for ic in range(ic_n):
            for n in range(N):
                for jc in range(4 * ic + 4):
                    specs.append((ic, n, jc))
        nspec = len(specs)
        qt_tiles = {0: qt_next}
        st2_of, at2_of, avs_of, qres_of = {}, {}, {}, {}
        filler = []
        quota = {}

        def emit_filler(k):
            for _ in range(k):
                if filler:
                    filler.pop(0)()

        def stage_scores(idx):
            ic, n, jc = specs[idx]
            st2 = st2p.tile([128, 1024], F32, tag="st2", name="st2")
            st2_of[idx] = st2
            qt_cur = qt_tiles[ic]
            for h in range(H2):
                nc.tensor.matmul(
                    st2[:, h * 512:(h + 1) * 512],
                    kt_sb[h * 64:(h + 1) * 64,
                          n * l + jc * 128:n * l + jc * 128 + 128],
                    qt_cur[h * 64:(h + 1) * 64, n * 512:(n + 1) * 512],
                    start=True, stop=True)

        def stage_exp(idx):
            ic, n, jc = specs[idx]
            rr = jc - 4 * ic
            st2 = st2_of.pop(idx)
            at2 = at2p.tile([128, 1024], BF16, tag="at2", name="at2")
            at2_of[idx] = at2
            if rr <= 0:
                nc.scalar.activation(at2, st2,
                                     mybir.ActivationFunctionType.Exp,
                                     scale=SCALE)
            else:
                for h in range(H2):
                    nc.scalar.activation(
                        at2[:, h * 512 + rr * 128:(h + 1) * 512],
                        st2[:, h * 512 + rr * 128:(h + 1) * 512],
                        mybir.ActivationFunctionType.Exp, scale=SCALE)

        def stage_mask(idx):
            ic, n, jc = specs[idx]
            rr = jc - 4 * ic
            if rr < 0:
                return
            at2 = at2_of[idx]
            for h in range(H2):
                sl = slice(h * 512 + rr * 128, h * 512 + rr * 128 + 128)
                nc.vector.tensor_mul(at2[:, sl], at2[:, sl], trimask)

        def stage_av(idx):
            ic, n, jc = specs[idx]
            rr = jc - 4 * ic
            at2 = at2_of.pop(idx)
            if jc == 0:
                avs_of[(ic, n)] = [avp.tile([128, 260], F32, tag=f"av{h}",
                                            name=f"av{h}")
                                   for h in range(H2)]
                for h in range(H2):
                    # clear the whole accumulator bank exactly once
                    nc.tensor.matmul(avs_of[(ic, n)][h], zlhs, zrhs,
                                     start=True, stop=True,
                                     skip_group_check=True)
            avs = avs_of[(ic, n)]
            vbase = (n * jblocks + jc) * H2 * 65
            for h in range(H2):
                for S in range(4):
                    if rr > S:
                        continue
                    nc.tensor.matmul(
                        avs[h][:, S * 65:(S + 1) * 65],
                        at2[:, h * 512 + S * 128:h * 512 + S * 128 + 128],
                        v_sb[:, vbase + h * 65:vbase + h * 65 + 65],
                        start=False, stop=(rr == S),
                        skip_group_check=True)
            if jc == 4 * ic + 3:
                enqueue_drain(ic, n)

        # drains and stats run as small deferred pieces, one per iteration,
        # so their engine dependencies are satisfied before dispatch and
        # they never block the in-order PE/DVE queues.
        drain_pending = []

        def enqueue_drain(ic, n):
            avs = avs_of.pop((ic, n))
            qres_t = qres_of.pop((ic, n))
            base512 = (n * 16 + ic * 4) * FC

            def drain_head(h):
                av3 = avs[h].rearrange("p (s x) -> p s x", x=65)
                rec = smallp.tile([128, 4], F32, tag="rec", name="rec")
                nc.vector.reciprocal(rec, av3[:, :, 64])
                for S in range(4):
                    nc.vector.scalar_tensor_tensor(
                        out=res_sb[:, base512 + S * FC + h * 64:
                                   base512 + S * FC + h * 64 + 64],
                        in0=avs[h][:, S * 65:S * 65 + 64],
                        scalar=rec[:, S:S + 1],
                        in1=qres_t[:, S * FC + h * 64:S * FC + h * 64 + 64],
                        op0=mybir.AluOpType.mult,
                        op1=mybir.AluOpType.add)

            def drain_sq():
                res_block = res_sb[:, base512:base512 + 512]
                sqt = sqp.tile([128, 512], BF16, tag="sq", name="sqt")
                nc.vector.tensor_mul(sqt, res_block, res_block)
                stats_bufs[(ic, n)] = (res_block, sqt)

            def drain_stats():
                res_block, sqt = stats_bufs.pop((ic, n))
                first = (n == 0 and ic == 0)
                last = (n == N - 1 and ic == ic_n - 1)
                nc.tensor.matmul(stat[0:1, :], ones_col, res_block,
                                 start=first, stop=last,
                                 skip_group_check=True)
                nc.tensor.matmul(stat[64:65, :], ones_col, sqt,
                                 start=first, stop=last,
                                 skip_group_check=True)

            if DEBUG_NOLAG:
                drain_head(0)
                drain_head(1)
                drain_sq()
                drain_stats()
            else:
                drain_pending.extend(
                    [lambda: drain_head(0), None,
                     lambda: (drain_head(1), drain_sq()), None, drain_stats])

        stats_bufs = {}

        def step_drain(flush=False):
            while drain_pending:
                op = drain_pending.pop(0)
                if op is None:
                    if flush:
                        continue
                    return
                op()

        # chunk 0's remaining projection groups (batches 1-3) are due just
        # before B(0) reaches that batch: group n spread over iters
        # [4(n-1), 4n)
        for n in range(1, N):
            ops = groups0[n]
            no = len(ops)
            for t in range(4):
                quota[4 * (n - 1) + t] = (quota.get(4 * (n - 1) + t, 0)
                                          + ((t + 1) * no) // 4
                                          - (t * no) // 4)
        filler = groups0[1] + groups0[2] + groups0[3]

        for idx in range(nspec + max(EXP_LAG, MASK_LAG, AV_LAG)):
            if idx < nspec:
                ic, n, jc = specs[idx]
                if jc == 0 and n == 0 and ic + 1 < ic_n:
                    # build next chunk's projection fillers, paced over
                    # this chunk's iterations (merged with any deadline
                    # quotas already scheduled for these slots)
                    if ic > 0:
                        emit_filler(len(filler))
                    qt_tiles[ic + 1] = qtp.tile([128, N * 512], BF16,
                                                tag="qt", name="qt")
                    nxq = load_x(xqp, xq_nd, 0, ic + 1)
                    nxk = load_x(xkp, xk_nd, 0, ic + 1)
                    groups = build_chunk_groups(ic + 1, qt_tiles[ic + 1],
                                                (nxq, nxk))
                    new_ops = [op for g in groups for op in g]
                    iters = N * (4 * ic + 4)
                    nf = len(new_ops)
                    for t in range(iters):
                        quota[idx + t] = (quota.get(idx + t, 0)
                                          + ((t + 1) * nf) // iters
                                          - (t * nf) // iters)
                    filler.extend(new_ops)
                if jc == 0:
                    qres_t = qinp.tile([128, 4 * FC], BF16, tag="qres",
                                       name="qres")
                    nc.sync.dma_start(
                        qres_t.rearrange("p (s f) -> p s f", s=4),
                        bass.AP(tensor=q_res.tensor,
                                offset=q_res.offset + (n * l + ic * 512) * FC,
                                ap=[[FC, 128], [128 * FC, 4], [1, FC]]))
                    qres_of[(ic, n)] = qres_t
                stage_scores(idx)
            if idx - EXP_LAG >= 0 and idx - EXP_LAG < nspec:
                stage_exp(idx - EXP_LAG)
            if idx - MASK_LAG >= 0 and idx - MASK_LAG < nspec:
                stage_mask(idx - MASK_LAG)
            step_drain()
            if idx - AV_LAG >= 0 and idx - AV_LAG < nspec:
                stage_av(idx - AV_LAG)
            emit_filler(quota.get(idx, 0))
            if idx == nspec - 1:
                emit_filler(len(filler))
        step_drain(flush=True)

        # ------------------------------------------------------------------
        # batch-norm: fold partial sums, compute gamma', beta', apply
        # ------------------------------------------------------------------
        # fold the 4 ls-group partials to [1, FC] on partition 0
        sum_r = bnp.tile([1, 512], F32, tag="sumr", name="sumr")
        nc.vector.tensor_copy(sum_r, stat[0:1, :])
        sq_r = bnp.tile([1, 512], F32, tag="sqr", name="sqr")
        nc.vector.tensor_copy(sq_r, stat[64:65, :])
        sA = bnp.tile([1, FC], F32, tag="sA", name="sA")
        nc.vector.tensor_add(sA, sum_r[:, 0:FC], sum_r[:, FC:2 * FC])
        sB = bnp.tile([1, FC], F32, tag="sB", name="sB")
        nc.vector.tensor_add(sB, sum_r[:, 2 * FC:3 * FC],
                             sum_r[:, 3 * FC:4 * FC])
        sumf = bnp.tile([1, FC], F32, tag="sumf", name="sumf")
        nc.vector.tensor_add(sumf, sA, sB)
        qA = bnp.tile([1, FC], F32, tag="qA", name="qA")
        nc.vector.tensor_add(qA, sq_r[:, 0:FC], sq_r[:, FC:2 * FC])
        qB = bnp.tile([1, FC], F32, tag="qB", name="qB")
        nc.vector.tensor_add(qB, sq_r[:, 2 * FC:3 * FC],
                             sq_r[:, 3 * FC:4 * FC])
        sqf = bnp.tile([1, FC], F32, tag="sqf", name="sqf")
        nc.vector.tensor_add(sqf, qA, qB)

        inv = 1.0 / NL
        mean = bnp.tile([1, FC], F32, tag="mean", name="mean")
        nc.vector.tensor_scalar_mul(mean, sumf, inv)
        musq = bnp.tile([1, FC], F32, tag="musq", name="musq")   # mean^2
        nc.vector.tensor_mul(musq, mean, mean)
        var = bnp.tile([1, FC], F32, tag="var", name="var")
        nc.vector.scalar_tensor_tensor(
            out=var, in0=sqf, scalar=inv, in1=musq,
            op0=mybir.AluOpType.mult, op1=mybir.AluOpType.subtract)
        std = bnp.tile([1, FC], F32, tag="std", name="std")
        nc.scalar.activation(std, var, mybir.ActivationFunctionType.Sqrt,
                             bias=eps_sb[0:1, :])
        rstd = bnp.tile([1, FC], F32, tag="rstd", name="rstd")
        nc.vector.reciprocal(rstd, std)
        gp = bnp.tile([1, FC], F32, tag="gp", name="gp")
        nc.vector.tensor_mul(gp, gamma_sb, rstd)
        mgp = bnp.tile([1, FC], F32, tag="mgp", name="mgp")
        nc.vector.tensor_mul(mgp, mean, gp)
        bp = bnp.tile([1, FC], F32, tag="bp", name="bp")
        nc.vector.tensor_sub(bp, beta_sb, mgp)
        gp16 = bnp.tile([1, FC], BF16, tag="gp16", name="gp16")
        nc.vector.tensor_copy(gp16, gp)
        bp16 = bnp.tile([1, FC], BF16, tag="bp16", name="bp16")
        nc.vector.tensor_copy(bp16, bp)

        gbc = bnp.tile([128, FC], BF16, tag="gbc", name="gbc")
        nc.gpsimd.partition_broadcast(gbc, gp16)
        bbc = bnp.tile([128, FC], BF16, tag="bbc", name="bbc")
        nc.gpsimd.partition_broadcast(bbc, bp16)

        def rep4(t):
            return bass.AP(tensor=t.tensor, offset=t.offset,
                           ap=[[t.ap[0][0], 128], [0, 4], [1, FC]])

        gbc4 = bnp.tile([128, 512], BF16, tag="gbc4", name="gbc4")
        nc.vector.tensor_copy(gbc4, rep4(gbc))
        bbc4 = bnp.tile([128, 512], BF16, tag="bbc4", name="bbc4")
        nc.vector.tensor_copy(bbc4, rep4(bbc))

        for n in range(N):
            for ic in range(ic_n):
                base512 = (n * 16 + ic * 4) * FC
                t1 = outp.tile([128, 512], BF16, tag="t1", name="t1")
                nc.vector.tensor_mul(t1, res_sb[:, base512:base512 + 512],
                                     gbc4)
                t2 = outp.tile([128, 512], BF16, tag="t2", name="t2")
                nc.vector.tensor_add(t2, t1, bbc4)
                nc.sync.dma_start(
                    bass.AP(tensor=out_s.tensor,
                            offset=out_s.offset + (n * l + ic * 512) * FC,
                            ap=[[FC, 128], [128 * FC, 4], [1, FC]]),
                    t2.rearrange("p (s f) -> p s f", s=4))

    nc.compile()
    return nc


def get_runner(nc):
    """Build (once) a cached jitted SPMD executor for the Bass program."""
    if "runner" in _cached:
        return _cached["runner"]

    import jax
    from jax.experimental.shard_map import shard_map
    from jax.sharding import Mesh, PartitionSpec
    from concourse import bass2jax

    bass2jax.install_neuronx_cc_hook()

    partition_name = (nc.partition_id_tensor.name
                      if nc.partition_id_tensor else None)
    in_names, out_names, out_avals, zero_outs = [], [], [], []
    for alloc in nc.m.functions[0].allocations:
        if not isinstance(alloc, mybir.MemoryLocationSet):
            continue
        name = alloc.memorylocations[0].name
        if alloc.kind == "ExternalInput":
            if name != partition_name:
                in_names.append(name)
        elif alloc.kind == "ExternalOutput":
            shape = tuple(alloc.tensor_shape)
            dtype = mybir.dt.np(alloc.dtype)
            out_names.append(name)
            out_avals.append(jax.core.ShapedArray(shape, dtype))
            zero_outs.append(np.zeros(shape, dtype))
    n_params = len(in_names)
    n_outs = len(out_avals)
    all_names = in_names + out_names
    if partition_name is not None:
        all_names = all_names + [partition_name]

    def _body(*args):
        operands = list(args)
        if partition_name is not None:
            operands.append(bass2jax.partition_id_tensor())
        outs = bass2jax._bass_exec_p.bind(
            *operands,
            out_avals=tuple(out_avals),
            in_names=tuple(all_names),
            out_names=tuple(out_names),
            lowering_input_output_aliases=(),
            sim_require_finite=True,
            sim_require_nnan=True,
            nc=nc,
        )
        return tuple(outs)

    devices = jax.devices()[:NCORES]
    mesh = Mesh(np.asarray(devices), ("core",))
    in_specs = (PartitionSpec("core"),) * (n_params + n_outs)
    out_specs = (PartitionSpec("core"),) * n_outs
    donate = tuple(range(n_params, n_params + n_outs))
    sharded = jax.jit(
        shard_map(_body, mesh=mesh, in_specs=in_specs, out_specs=out_specs,
                  check_rep=False),
        donate_argnums=donate, keep_unused=True)

    def run_np(in_maps):
        concat_in = [
            np.concatenate([np.asarray(in_maps[c][nm]) for c in range(NCORES)],
                           axis=0)
            for nm in in_names]
        concat_zeros = [np.zeros((NCORES * z.shape[0], *z.shape[1:]), z.dtype)
                        for z in zero_outs]
        out_arrs = sharded(*concat_in, *concat_zeros)
        return [
            {nm: np.asarray(out_arrs[i]).reshape(
                NCORES, *out_avals[i].shape)[c]
             for i, nm in enumerate(out_names)}
            for c in range(NCORES)]

    _cached["runner"] = (run_np, sharded, in_names, out_names, out_avals,
                         zero_outs, mesh)
    return _cached["runner"]


def make_in_maps(inputs, l):
    query = np.asarray(inputs["query"], dtype=np.float32)
    key = np.asarray(inputs["key"], dtype=np.float32)
    Wq = np.asarray(inputs["Wq"], dtype=np.float32)
    Wk = np.asarray(inputs["Wk"], dtype=np.float32)
    Wv = np.asarray(inputs["Wv"], dtype=np.float32)
    gamma = np.asarray(inputs["gamma"], dtype=np.float32)
    beta = np.asarray(inputs["beta"], dtype=np.float32)

    n = query.shape[0]
    qf = query.reshape(n * l, D)
    kf = key.reshape(n * l, D)
    xq = np.ascontiguousarray(qf.T.astype(BF16_NP))
    xk = np.ascontiguousarray(kf.T.astype(BF16_NP))

    in_maps = []
    for c in range(NCORES):
        sl = slice(c * FC, (c + 1) * FC)
        in_maps.append({
            "xq_nd": xq,
            "xk_nd": xk,
            "wqt": np.ascontiguousarray(Wq[sl].T.astype(BF16_NP)),
            "wkt": np.ascontiguousarray(Wk[sl].T.astype(BF16_NP)),
            "wvt": np.ascontiguousarray(Wv[sl].T.astype(BF16_NP)),
            "q_res": np.ascontiguousarray(qf[:, sl].astype(BF16_NP)),
            "gamma": np.ascontiguousarray(gamma[sl].reshape(1, FC)),
            "beta": np.ascontiguousarray(beta[sl].reshape(1, FC)),
        })
    return in_maps


def kernel(**inputs):
    l = np.asarray(inputs["query"]).shape[1]
    if "nc" not in _cached or _cached.get("l") != l:
        _cached["nc"] = build_program(l)
        _cached["l"] = l
    nc = _cached["nc"]

    in_maps = make_in_maps(inputs, l)
    run_np = get_runner(nc)[0]
    results = run_np(in_maps)

    n = np.asarray(inputs["query"]).shape[0]
    out = np.zeros((n, l, D), dtype=np.float32)
    for c in range(NCORES):
        sl = slice(c * FC, (c + 1) * FC)
        out[:, :, sl] = results[c]["out_s"].reshape(n, l, FC).astype(
            np.float32)
    return out


# revision 46
# speedup vs baseline: 1.0045x; 1.0045x over previous
"""Trainium2 Bass kernel for MultiHeadAttention + residual + BatchNorm.

Model (reference):
  q = query @ Wq.T ; k = key @ Wk.T ; v = key @ Wv.T    (per-head split)
  score = q k^T / sqrt(D), causal mask, softmax over keys
  res   = (attn @ v) + query
  out   = batchnorm(res over all (N*L) rows, per feature) * gamma + beta

Sharding over 8 cores: FEATURE sharding. Core c owns heads {2c, 2c+1}
(features [128c, 128c+128)) for ALL batches. BatchNorm statistics are
then core-local (sums over all N*L rows of the core's own features), so
no collective is needed at all.

All matmul operands are bf16 (PSUM accumulation stays fp32); the
residual add, batch-norm statistics and outputs are fp32.

Per 512-row query chunk ic and batch n the attention inner loop walks
key blocks jc (128 keys each, causal): PE computes both heads' scores
into one PSUM tile [128 j, 1024 (h,i)], ScalarE applies a single merged
exp -> bf16, DVE masks the diagonal block with a triangular multiply,
and PE accumulates the *flipped* attention-V product out[i, p] with a
ones-column appended to V so softmax denominators fall out of the same
matmuls. Projections for chunk ic+1 are emitted as small PE micro-ops
paced between attention iterations so the scalar engine never starves.
"""

import math
import sys

sys.path.insert(0, "/opt/trn_rl_repo")

import numpy as np
import ml_dtypes

import concourse.bass as bass
import concourse.mybir as mybir
from concourse import bacc
import concourse.tile as tile

F32 = mybir.dt.float32
F32R = mybir.dt.float32r
BF16 = mybir.dt.bfloat16
BF16_NP = ml_dtypes.bfloat16

N = 4
L = 2048
D = 1024
H = 16
P = 64
NCORES = 8
FC = D // NCORES       # features per core = 128
H2 = 2                 # heads per core
EPS = 1e-5
SCALE = 1.0 / math.sqrt(D)
NL = N * L             # 8192 rows in the global batch norm

# software-pipeline stage lags (consumers trail producers so engine wait
# queues never backpressure the sequencers)
EXP_LAG = 1
MASK_LAG = 2
AV_LAG = 3
DEBUG_NOLAG = False

_cached = {}


def r(ap):
    return ap.bitcast(F32R)


def build_program(l=L):
    """Build the SPMD Bass program (identical on all 8 cores)."""
    nc = bacc.Bacc("TRN2", target_bir_lowering=False, debug=False,
                   num_devices=NCORES)

    ic_n = l // 512        # 512-row query chunks
    nlc = N * l            # rows per core (all batches)

    xq_nd = nc.dram_tensor("xq_nd", [D, nlc], BF16, kind="ExternalInput").ap()
    xk_nd = nc.dram_tensor("xk_nd", [D, nlc], BF16, kind="ExternalInput").ap()
    wqt = nc.dram_tensor("wqt", [D, FC], BF16, kind="ExternalInput").ap()
    wkt = nc.dram_tensor("wkt", [D, FC], BF16, kind="ExternalInput").ap()
    wvt = nc.dram_tensor("wvt", [D, FC], BF16, kind="ExternalInput").ap()
    q_res = nc.dram_tensor("q_res", [nlc, FC], BF16, kind="ExternalInput").ap()
    gamma = nc.dram_tensor("gamma", [1, FC], F32, kind="ExternalInput").ap()
    beta = nc.dram_tensor("beta", [1, FC], F32, kind="ExternalInput").ap()
    out_s = nc.dram_tensor("out_s", [nlc, FC], BF16,
                           kind="ExternalOutput").ap()

    jblocks = l // 128     # 16 key blocks per batch
    nls = nlc // 128       # 64 ls blocks of res

    from contextlib import ExitStack
    with ExitStack() as stack:
        tc = stack.enter_context(tile.TileContext(nc))
        pool = {}
        for nm, bufs, space in (
                ("consts", 1, None), ("persist", 1, None), ("wt", 1, None),
                ("qtp", 2, None), ("xq", 2, None), ("xk", 2, None),
                ("at2", 3, None), ("qin", 2, None), ("sq", 2, None),
                ("outp", 3, None), ("bnp", 1, None), ("small", 6, None),
                ("st2", 2, "PSUM"), ("av", 1, "PSUM"), ("pj", 1, "PSUM"),
                ("stat", 1, "PSUM")):
            kw = {"name": nm, "bufs": bufs}
            if space:
                kw["space"] = space
            pool[nm] = stack.enter_context(tc.tile_pool(**kw))
        consts, persist, wtp = pool["consts"], pool["persist"], pool["wt"]
        qtp, xqp, xkp = pool["qtp"], pool["xq"], pool["xk"]
        at2p, qinp, sqp = pool["at2"], pool["qin"], pool["sq"]
        outp, bnp, smallp = pool["outp"], pool["bnp"], pool["small"]
        st2p, avp, pjp, statp = (pool["st2"], pool["av"], pool["pj"],
                                 pool["stat"])

        # ---------------- first activation chunk DMAs (critical path) ----
        def x_src(dram, n, ic):
            return bass.AP(
                tensor=dram.tensor,
                offset=dram.offset + n * l + ic * 512,
                ap=[[nlc, 128], [128 * nlc, 8], [1, 512]])

        def load_x(pool, dram, n, ic):
            t = pool.tile([128, 8 * 512], BF16, tag="x", name="xt")
            nc.sync.dma_start(
                t.rearrange("p (dc x) -> p dc x", dc=8), x_src(dram, n, ic))
            return t

        # first q-projection chain needs wq then xq: emit those two DMAs
        # first so PE can start as early as possible
        wts = {}

        def load_w(wname, wdram):
            t = wtp.tile([128, 8 * FC], BF16, tag=wname, name=wname)
            nc.sync.dma_start(
                t.rearrange("p (dc f) -> p dc f", dc=8),
                bass.AP(tensor=wdram.tensor, offset=wdram.offset,
                        ap=[[FC, 128], [128 * FC, 8], [1, FC]]))
            wts[wname] = t

        load_w("wq", wqt)
        xq_t = load_x(xqp, xq_nd, 0, 0)
        load_w("wk", wkt)
        xk_t = load_x(xkp, xk_nd, 0, 0)
        load_w("wv", wvt)

        # ---------------- constants -------------------------------------
        ones_col = consts.tile([128, 1], BF16)
        nc.vector.memset(ones_col, 1.0)
        eps_sb = consts.tile([128, 1], F32)
        nc.vector.memset(eps_sb, EPS)
        gamma_sb = consts.tile([1, FC], F32)
        nc.sync.dma_start(gamma_sb, gamma)
        beta_sb = consts.tile([1, FC], F32)
        nc.sync.dma_start(beta_sb, beta)
        # lower-triangular (j <= i) mask in [j-part, i-free] layout
        tm_f = consts.tile([128, 128], F32)
        nc.vector.memset(tm_f, 1.0)
        nc.gpsimd.affine_select(
            out=tm_f, in_=tm_f,
            compare_op=mybir.AluOpType.is_ge, fill=0.0, base=0,
            pattern=[[1, 128]], channel_multiplier=-1)
        trimask = consts.tile([128, 128], BF16)
        nc.vector.tensor_copy(trimask, tm_f)
        # zero operands for the PSUM-bank-clearing matmuls (PSUM
        # start_tensor_calc marks a whole 2KB zero-region, so banks shared
        # by several accumulation groups must be cleared by one explicit
        # full-tile start matmul instead of per-group start flags)
        zlhs = consts.tile([128, 128], BF16)
        nc.vector.memset(zlhs, 0.0)
        zrhs = consts.tile([128, 260], BF16)
        nc.vector.memset(zrhs, 0.0)

        # ---------------- persistent SBUF -------------------------------
        # kt_sb: [feature(h*64+p), (n, j)] bf16
        kt_sb = persist.tile([128, N * l], BF16, tag="kt")
        # v_sb: [j-in-block, (n, jc, h, 65)] bf16; col 64 of each 65-group
        # is the baked ones column (softmax denominator trick)
        v_sb = persist.tile([128, N * jblocks * H2 * 65], BF16, tag="v")
        v3 = v_sb.rearrange("p (g x) -> p g x", x=65)
        nc.gpsimd.memset(v3[:, :, 64:65], 1.0)
        # res_sb: [l-in-block, (ls, f)] bf16, ls = n*16 + ic*4 + S
        res_sb = persist.tile([128, nls * FC], BF16, tag="res")

        # partition 0 row: sums; partition 64 row: sums of squares
        # (matmul outputs must start at partition 0, 32, or 64)
        stat = statp.tile([65, 512], F32, tag="stat")

        # ------------------------------------------------------------------
        # projection task machinery (filler micro-ops paced into B loops)
        # ------------------------------------------------------------------
        # rotating PSUM allocators: the interleaved fillers use the single
        # pj bank; the up-front A(0) block also rotates through the two
        # (then idle) score banks so chains overlap their copy-out
        def alloc_pj():
            return pjp.tile([128, 512], F32, tag="pj", name="pj")

        def alloc_st2_slot():
            return st2p.tile([128, 1024], F32, tag="st2", name="st2")[:, 0:512]

        rot = {"i": 0}

        def alloc_rotating():
            rot["i"] += 1
            return alloc_pj() if rot["i"] % 3 == 0 else alloc_st2_slot()

        chain_alloc = {"fn": alloc_pj}

        def qk_chain(side, n, ic, qt_tile, get_xt):
            """Emit one q/k projection chain as a list of micro closures."""
            w_use = wts["wq"] if side == "q" else wts["wk"]
            pj = {}
            alloc_fn = chain_alloc["fn"]

            def alloc():
                pj["t"] = alloc_fn()

            def mm(dc0):
                xt = get_xt()
                for dc in (dc0, dc0 + 1):
                    nc.tensor.matmul(
                        pj["t"],
                        wts_slice(w_use, dc),
                        xt[:, dc * 512:(dc + 1) * 512],
                        start=(dc == 0), stop=(dc == 7))

            def copy():
                if side == "q":
                    nc.vector.tensor_copy(qt_tile[:, n * 512:(n + 1) * 512],
                                          pj["t"])
                else:
                    nc.vector.tensor_copy(
                        kt_sb[:, n * l + ic * 512:n * l + ic * 512 + 512],
                        pj["t"])

            ops = [alloc]
            for dc0 in range(0, 8, 2):
                ops.append(lambda d=dc0: mm(d))
            ops.append(copy)
            return ops

        def wts_slice(w, dc):
            return w[:, dc * FC:(dc + 1) * FC]

        def v_chain(n, jsub, ic, get_xt):
            pj = {}
            alloc_fn = chain_alloc["fn"]

            def alloc():
                pj["t"] = alloc_fn()

            def mm(dc0):
                xt = get_xt()
                for dc in (dc0, dc0 + 1):
                    nc.tensor.matmul(
                        pj["t"][:, 0:128],
                        xt[:, dc * 512 + jsub * 128:dc * 512 + jsub * 128 + 128],
                        wts_slice(wts["wv"], dc),
                        start=(dc == 0), stop=(dc == 7))

            def copy():
                jc = ic * 4 + jsub
                base = (n * jblocks + jc) * H2 * 65
                dst = v_sb[:, base:base + 130].rearrange(
                    "p (h x) -> p h x", h=2)[:, :, 0:64]
                src = pj["t"][:, 0:128].rearrange("p (h x) -> p h x", h=2)
                nc.vector.tensor_copy(dst, src)

            ops = [alloc]
            for dc0 in range(0, 8, 2):
                ops.append(lambda d=dc0: mm(d))
            ops.append(copy)
            return ops

        def build_chunk_groups(ic, qt_tile, first_x, rotate_first=False):
            """Return per-batch lists of micro-op closures for A(ic)."""
            groups = []
            xq_cur = {0: first_x[0]}
            xk_cur = {0: first_x[1]}
            for n in range(N):
                chain_alloc["fn"] = (alloc_rotating if rotate_first and n == 0
                                     else alloc_pj)
                ops = []
                get_xq = lambda nn=n: xq_cur[nn]
                get_xk = lambda nn=n: xk_cur[nn]
                # prefetch next batch's activations
                if n + 1 < N:
                    def pre(nn=n + 1):
                        xq_cur[nn] = load_x(xqp, xq_nd, nn, ic)
                        xk_cur[nn] = load_x(xkp, xk_nd, nn, ic)
                    ops.append(pre)
                ops += qk_chain("q", n, ic, qt_tile, get_xq)
                ops += qk_chain("k", n, ic, None, get_xk)
                for jsub in range(4):
                    ops += v_chain(n, jsub, ic, get_xk)
                groups.append(ops)
            return groups

        # ------------------------------------------------------------------
        # A(0): only batch 0's projections run up front; batches 1-3 are
        # deadline-paced into B(0)'s iterations (batch n is needed at
        # B(0) iteration 4n) so the scalar engine starts exp'ing early.
        # ------------------------------------------------------------------
        qt_next = qtp.tile([128, N * 512], BF16, tag="qt", name="qt")
        groups0 = build_chunk_groups(0, qt_next, (xq_t, xk_t),
                                     rotate_first=True)
        for op in groups0[0]:
            op()

        # ------------------------------------------------------------------
        # main loop: one software-pipelined stream over (ic, n, jc).
        # Stage schedule at step t: scores(t), exp(t-1), mask(t-2), AV(t-3)
        # so every instruction's inputs are ready when the engine decodes
        # it (the 4-deep per-engine wait queues otherwise backpressure the
        # sequencers). A(ic+1) projection micro-ops are paced in as filler.
        # ------------------------------------------------------------------
        specs = []
        for ic in range(ic_n):
            for n in range(N):
                for jc in range(4 * ic + 4):
                    specs.append((ic, n, jc))
        nspec = len(specs)
        qt_tiles = {0: qt_next}
        st2_of, at2_of, avs_of, qres_of = {}, {}, {}, {}
        filler = []
        quota = {}

        def emit_filler(k):
            for _ in range(k):
                if filler:
                    filler.pop(0)()

        def stage_scores(idx):
            ic, n, jc = specs[idx]
            st2 = st2p.tile([128, 1024], F32, tag="st2", name="st2")
            st2_of[idx] = st2
            qt_cur = qt_tiles[ic]
            for h in range(H2):
                nc.tensor.matmul(
                    st2[:, h * 512:(h + 1) * 512],
                    kt_sb[h * 64:(h + 1) * 64,
                          n * l + jc * 128:n * l + jc * 128 + 128],
                    qt_cur[h * 64:(h + 1) * 64, n * 512:(n + 1) * 512],
                    start=True, stop=True)

        def stage_exp(idx):
            ic, n, jc = specs[idx]
            rr = jc - 4 * ic
            st2 = st2_of.pop(idx)
            at2 = at2p.tile([128, 1024], BF16, tag="at2", name="at2")
            at2_of[idx] = at2
            if rr <= 0:
                nc.scalar.activation(at2, st2,
                                     mybir.ActivationFunctionType.Exp,
                                     scale=SCALE)
            else:
                for h in range(H2):
                    nc.scalar.activation(
                        at2[:, h * 512 + rr * 128:(h + 1) * 512],
                        st2[:, h * 512 + rr * 128:(h + 1) * 512],
                        mybir.ActivationFunctionType.Exp, scale=SCALE)

        def stage_mask(idx):
            ic, n, jc = specs[idx]
            rr = jc - 4 * ic
            if rr < 0:
                return
            at2 = at2_of[idx]
            for h in range(H2):
                sl = slice(h * 512 + rr * 128, h * 512 + rr * 128 + 128)
                nc.vector.tensor_mul(at2[:, sl], at2[:, sl], trimask)

        def stage_av(idx):
            ic, n, jc = specs[idx]
            rr = jc - 4 * ic
            at2 = at2_of.pop(idx)
            if jc == 0:
                avs_of[(ic, n)] = [avp.tile([128, 260], F32, tag=f"av{h}",
                                            name=f"av{h}")
                                   for h in range(H2)]
                for h in range(H2):
                    # clear the whole accumulator bank exactly once
                    nc.tensor.matmul(avs_of[(ic, n)][h], zlhs, zrhs,
                                     start=True, stop=True,
                                     skip_group_check=True)
            avs = avs_of[(ic, n)]
            vbase = (n * jblocks + jc) * H2 * 65
            for h in range(H2):
                for S in range(4):
                    if rr > S:
                        continue
                    nc.tensor.matmul(
                        avs[h][:, S * 65:(S + 1) * 65],
                        at2[:, h * 512 + S * 128:h * 512 + S * 128 + 128],
                        v_sb[:, vbase + h * 65:vbase + h * 65 + 65],
                        start=False, stop=(rr == S),
                        skip_group_check=True)
            if jc == 4 * ic + 3:
                enqueue_drain(ic, n)

        # drains and stats run as small deferred pieces, one per iteration,
        # so their engine dependencies are satisfied before dispatch and
        # they never block the in-order PE/DVE queues.
        drain_pending = []

        def enqueue_drain(ic, n):
            avs = avs_of.pop((ic, n))
            qres_t = qres_of.pop((ic, n))
            base512 = (n * 16 + ic * 4) * FC

            def drain_head(h):
                av3 = avs[h].rearrange("p (s x) -> p s x", x=65)
                rec = smallp.tile([128, 4], F32, tag="rec", name="rec")
                nc.vector.reciprocal(rec, av3[:, :, 64])
                for S in range(4):
                    nc.vector.scalar_tensor_tensor(
                        out=res_sb[:, base512 + S * FC + h * 64:
                                   base512 + S * FC + h * 64 + 64],
                        in0=avs[h][:, S * 65:S * 65 + 64],
                        scalar=rec[:, S:S + 1],
                        in1=qres_t[:, S * FC + h * 64:S * FC + h * 64 + 64],
                        op0=mybir.AluOpType.mult,
                        op1=mybir.AluOpType.add)

            def drain_sq():
                res_block = res_sb[:, base512:base512 + 512]
                sqt = sqp.tile([128, 512], BF16, tag="sq", name="sqt")
                nc.vector.tensor_mul(sqt, res_block, res_block)
                stats_bufs[(ic, n)] = (res_block, sqt)

            def drain_stats():
                res_block, sqt = stats_bufs.pop((ic, n))
                first = (n == 0 and ic == 0)
                last = (n == N - 1 and ic == ic_n - 1)
                nc.tensor.matmul(stat[0:1, :], ones_col, res_block,
                                 start=first, stop=last,
                                 skip_group_check=True)
                nc.tensor.matmul(stat[64:65, :], ones_col, sqt,
                                 start=first, stop=last,
                                 skip_group_check=True)

            if DEBUG_NOLAG:
                drain_head(0)
                drain_head(1)
                drain_sq()
                drain_stats()
            else:
                drain_pending.extend(
                    [lambda: drain_head(0), None,
                     lambda: (drain_head(1), drain_sq()), None, drain_stats])

        stats_bufs = {}

        def step_drain(flush=False):
            while drain_pending:
                op = drain_pending.pop(0)
                if op is None:
                    if flush:
                        continue
                    return
                op()

        # chunk 0's remaining projection groups (batches 1-3) are due just
        # before B(0) reaches that batch: group n spread over iters
        # [4(n-1), 4n)
        for n in range(1, N):
            ops = groups0[n]
            no = len(ops)
            for t in range(4):
                quota[4 * (n - 1) + t] = (quota.get(4 * (n - 1) + t, 0)
                                          + ((t + 1) * no) // 4
                                          - (t * no) // 4)
        filler = groups0[1] + groups0[2] + groups0[3]

        for idx in range(nspec + max(EXP_LAG, MASK_LAG, AV_LAG)):
            if idx < nspec:
                ic, n, jc = specs[idx]
                if jc == 0 and n == 0 and ic + 1 < ic_n:
                    # build next chunk's projection fillers, paced over
                    # this chunk's iterations (merged with any deadline
                    # quotas already scheduled for these slots)
                    if ic > 0:
                        emit_filler(len(filler))
                    qt_tiles[ic + 1] = qtp.tile([128, N * 512], BF16,
                                                tag="qt", name="qt")
                    nxq = load_x(xqp, xq_nd, 0, ic + 1)
                    nxk = load_x(xkp, xk_nd, 0, ic + 1)
                    groups = build_chunk_groups(ic + 1, qt_tiles[ic + 1],
                                                (nxq, nxk))
                    new_ops = [op for g in groups for op in g]
                    iters = N * (4 * ic + 4)
                    nf = len(new_ops)
                    for t in range(iters):
                        quota[idx + t] = (quota.get(idx + t, 0)
                                          + ((t + 1) * nf) // iters
                                          - (t * nf) // iters)
                    filler.extend(new_ops)
                if jc == 0:
                    qres_t = qinp.tile([128, 4 * FC], BF16, tag="qres",
                                       name="qres")
                    nc.sync.dma_start(
                        qres_t.rearrange("p (s f) -> p s f", s=4),
                        bass.AP(tensor=q_res.tensor,
                                offset=q_res.offset + (n * l + ic * 512) * FC,
                                ap=[[FC, 128], [128 * FC, 4], [1, FC]]))
                    qres_of[(ic, n)] = qres_t
                stage_scores(idx)
            if idx - EXP_LAG >= 0 and idx - EXP_LAG < nspec:
                stage_exp(idx - EXP_LAG)
            if idx - MASK_LAG >= 0 and idx - MASK_LAG < nspec:
                stage_mask(idx - MASK_LAG)
            step_drain()
            if idx - AV_LAG >= 0 and idx - AV_LAG < nspec:
                stage_av(idx - AV_LAG)
            emit_filler(quota.get(idx, 0))
            if idx == nspec - 1:
                emit_filler(len(filler))
        step_drain(flush=True)

        # ------------------------------------------------------------------
        # batch-norm: fold partial sums, compute gamma', beta', apply
        # ------------------------------------------------------------------
        # fold the 4 ls-group partials to [1, FC] on partition 0
        sum_r = bnp.tile([1, 512], F32, tag="sumr", name="sumr")
        nc.vector.tensor_copy(sum_r, stat[0:1, :])
        sq_r = bnp.tile([1, 512], F32, tag="sqr", name="sqr")
        nc.vector.tensor_copy(sq_r, stat[64:65, :])
        sA = bnp.tile([1, FC], F32, tag="sA", name="sA")
        nc.vector.tensor_add(sA, sum_r[:, 0:FC], sum_r[:, FC:2 * FC])
        sB = bnp.tile([1, FC], F32, tag="sB", name="sB")
        nc.vector.tensor_add(sB, sum_r[:, 2 * FC:3 * FC],
                             sum_r[:, 3 * FC:4 * FC])
        sumf = bnp.tile([1, FC], F32, tag="sumf", name="sumf")
        nc.vector.tensor_add(sumf, sA, sB)
        qA = bnp.tile([1, FC], F32, tag="qA", name="qA")
        nc.vector.tensor_add(qA, sq_r[:, 0:FC], sq_r[:, FC:2 * FC])
        qB = bnp.tile([1, FC], F32, tag="qB", name="qB")
        nc.vector.tensor_add(qB, sq_r[:, 2 * FC:3 * FC],
                             sq_r[:, 3 * FC:4 * FC])
        sqf = bnp.tile([1, FC], F32, tag="sqf", name="sqf")
        nc.vector.tensor_add(sqf, qA, qB)

        inv = 1.0 / NL
        mean = bnp.tile([1, FC], F32, tag="mean", name="mean")
        nc.vector.tensor_scalar_mul(mean, sumf, inv)
        musq = bnp.tile([1, FC], F32, tag="musq", name="musq")   # mean^2
        nc.vector.tensor_mul(musq, mean, mean)
        var = bnp.tile([1, FC], F32, tag="var", name="var")
        nc.vector.scalar_tensor_tensor(
            out=var, in0=sqf, scalar=inv, in1=musq,
            op0=mybir.AluOpType.mult, op1=mybir.AluOpType.subtract)
        std = bnp.tile([1, FC], F32, tag="std", name="std")
        nc.scalar.activation(std, var, mybir.ActivationFunctionType.Sqrt,
                             bias=eps_sb[0:1, :])
        rstd = bnp.tile([1, FC], F32, tag="rstd", name="rstd")
        nc.vector.reciprocal(rstd, std)
        gp = bnp.tile([1, FC], F32, tag="gp", name="gp")
        nc.vector.tensor_mul(gp, gamma_sb, rstd)
        mgp = bnp.tile([1, FC], F32, tag="mgp", name="mgp")
        nc.vector.tensor_mul(mgp, mean, gp)
        bp = bnp.tile([1, FC], F32, tag="bp", name="bp")
        nc.vector.tensor_sub(bp, beta_sb, mgp)
        gp16 = bnp.tile([1, FC], BF16, tag="gp16", name="gp16")
        nc.vector.tensor_copy(gp16, gp)
        bp16 = bnp.tile([1, FC], BF16, tag="bp16", name="bp16")
        nc.vector.tensor_copy(bp16, bp)

        gbc = bnp.tile([128, FC], BF16, tag="gbc", name="gbc")
        nc.gpsimd.partition_broadcast(gbc, gp16)
        bbc = bnp.tile([128, FC], BF16, tag="bbc", name="bbc")
        nc.gpsimd.partition_broadcast(bbc, bp16)

        def rep4(t):
            return bass.AP(tensor=t.tensor, offset=t.offset,
                           ap=[[t.ap[0][0], 128], [0, 4], [1, FC]])

        gbc4 = bnp.tile([128, 512], BF16, tag="gbc4", name="gbc4")
        nc.vector.tensor_copy(gbc4, rep4(gbc))
        bbc4 = bnp.tile([128, 512], BF16, tag="bbc4", name="bbc4")
        nc.vector.tensor_copy(bbc4, rep4(bbc))

        for n in range(N):
            for ic in range(ic_n):
                base512 = (n * 16 + ic * 4) * FC
                t1 = outp.tile([128, 512], BF16, tag="t1", name="t1")
                nc.vector.tensor_mul(t1, res_sb[:, base512:base512 + 512],
                                     gbc4)
                t2 = outp.tile([128, 512], BF16, tag="t2", name="t2")
                nc.vector.tensor_add(t2, t1, bbc4)
                nc.sync.dma_start(
                    bass.AP(tensor=out_s.tensor,
                            offset=out_s.offset + (n * l + ic * 512) * FC,
                            ap=[[FC, 128], [128 * FC, 4], [1, FC]]),
                    t2.rearrange("p (s f) -> p s f", s=4))

    nc.compile()
    return nc


def get_runner(nc):
    """Build (once) a cached jitted SPMD executor for the Bass program."""
    if "runner" in _cached:
        return _cached["runner"]

    import jax
    from jax.experimental.shard_map import shard_map
    from jax.sharding import Mesh, PartitionSpec
    from concourse import bass2jax

    bass2jax.install_neuronx_cc_hook()

    partition_name = (nc.partition_id_tensor.name
                      if nc.partition_id_tensor else None)
    in_names, out_names, out_avals, zero_outs = [], [], [], []
    for alloc in nc.m.functions[0].allocations:
        if not isinstance(alloc, mybir.MemoryLocationSet):
            continue
        name = alloc.memorylocations[0].name
        if alloc.kind == "ExternalInput":
            if name != partition_name:
                in_names.append(name)
        elif alloc.kind == "ExternalOutput":
            shape = tuple(alloc.tensor_shape)
            dtype = mybir.dt.np(alloc.dtype)
            out_names.append(name)
            out_avals.append(jax.core.ShapedArray(shape, dtype))
            zero_outs.append(np.zeros(shape, dtype))
    n_params = len(in_names)
    n_outs = len(out_avals)
    all_names = in_names + out_names
    if partition_name is not None:
        all_names = all_names + [partition_name]

    def _body(*args):
        operands = list(args)
        if partition_name is not None:
            operands.append(bass2jax.partition_id_tensor())
        outs = bass2jax._bass_exec_p.bind(
            *operands,
            out_avals=tuple(out_avals),
            in_names=tuple(all_names),
            out_names=tuple(out_names),
            lowering_input_output_aliases=(),
            sim_require_finite=True,
            sim_require_nnan=True,
            nc=nc,
        )
        return tuple(outs)

    devices = jax.devices()[:NCORES]
    mesh = Mesh(np.asarray(devices), ("core",))
    in_specs = (PartitionSpec("core"),) * (n_params + n_outs)
    out_specs = (PartitionSpec("core"),) * n_outs
    donate = tuple(range(n_params, n_params + n_outs))
    sharded = jax.jit(
        shard_map(_body, mesh=mesh, in_specs=in_specs, out_specs=out_specs,
                  check_rep=False),
        donate_argnums=donate, keep_unused=True)

    def run_np(in_maps):
        concat_in = [
            np.concatenate([np.asarray(in_maps[c][nm]) for c in range(NCORES)],
                           axis=0)
            for nm in in_names]
        concat_zeros = [np.zeros((NCORES * z.shape[0], *z.shape[1:]), z.dtype)
                        for z in zero_outs]
        out_arrs = sharded(*concat_in, *concat_zeros)
        return [
            {nm: np.asarray(out_arrs[i]).reshape(
                NCORES, *out_avals[i].shape)[c]
             for i, nm in enumerate(out_names)}
            for c in range(NCORES)]

    _cached["runner"] = (run_np, sharded, in_names, out_names, out_avals,
                         zero_outs, mesh)
    return _cached["runner"]


def make_in_maps(inputs, l):
    query = np.asarray(inputs["query"], dtype=np.float32)
    key = np.asarray(inputs["key"], dtype=np.float32)
    Wq = np.asarray(inputs["Wq"], dtype=np.float32)
    Wk = np.asarray(inputs["Wk"], dtype=np.float32)
    Wv = np.asarray(inputs["Wv"], dtype=np.float32)
    gamma = np.asarray(inputs["gamma"], dtype=np.float32)
    beta = np.asarray(inputs["beta"], dtype=np.float32)

    n = query.shape[0]
    qf = query.reshape(n * l, D)
    kf = key.reshape(n * l, D)
    xq = np.ascontiguousarray(qf.T.astype(BF16_NP))
    xk = np.ascontiguousarray(kf.T.astype(BF16_NP))

    in_maps = []
    for c in range(NCORES):
        sl = slice(c * FC, (c + 1) * FC)
        in_maps.append({
            "xq_nd": xq,
            "xk_nd": xk,
            "wqt": np.ascontiguousarray(Wq[sl].T.astype(BF16_NP)),
            "wkt": np.ascontiguousarray(Wk[sl].T.astype(BF16_NP)),
            "wvt": np.ascontiguousarray(Wv[sl].T.astype(BF16_NP)),
            "q_res": np.ascontiguousarray(qf[:, sl].astype(BF16_NP)),
            "gamma": np.ascontiguousarray(gamma[sl].reshape(1, FC)),
            "beta": np.ascontiguousarray(beta[sl].reshape(1, FC)),
        })
    return in_maps


def kernel(**inputs):
    l = np.asarray(inputs["query"]).shape[1]
    if "nc" not in _cached or _cached.get("l") != l:
        _cached["nc"] = build_program(l)
        _cached["l"] = l
    nc = _cached["nc"]

    in_maps = make_in_maps(inputs, l)
    run_np = get_runner(nc)[0]
    results = run_np(in_maps)

    n = np.asarray(inputs["query"]).shape[0]
    out = np.zeros((n, l, D), dtype=np.float32)
    for c in range(NCORES):
        sl = slice(c * FC, (c + 1) * FC)
        out[:, :, sl] = results[c]["out_s"].reshape(n, l, FC).astype(
            np.float32)
    return out


# revision 56
# speedup vs baseline: 1.1429x; 1.1378x over previous
"""Trainium2 Bass kernel for MultiHeadAttention + residual + BatchNorm.

Model (reference):
  q = query @ Wq.T ; k = key @ Wk.T ; v = key @ Wv.T    (per-head split)
  score = q k^T / sqrt(D), causal mask, softmax over keys
  res   = (attn @ v) + query
  out   = batchnorm(res over all (N*L) rows, per feature) * gamma + beta

Sharding over 8 cores: FEATURE sharding. Core c owns heads {2c, 2c+1}
(features [128c, 128c+128)) for ALL batches. BatchNorm statistics are
then core-local (sums over all N*L rows of the core's own features), so
no collective is needed at all.

All matmul operands are bf16 (PSUM accumulation stays fp32); the
residual add, batch-norm statistics and outputs are fp32.

Per 512-row query chunk ic and batch n the attention inner loop walks
key blocks jc (128 keys each, causal): PE computes both heads' scores
into one PSUM tile [128 j, 1024 (h,i)], ScalarE applies a single merged
exp -> bf16, DVE masks the diagonal block with a triangular multiply,
and PE accumulates the *flipped* attention-V product out[i, p] with a
ones-column appended to V so softmax denominators fall out of the same
matmuls. Projections for chunk ic+1 are emitted as small PE micro-ops
paced between attention iterations so the scalar engine never starves.
"""

import math
import sys

sys.path.insert(0, "/opt/trn_rl_repo")

import numpy as np
import ml_dtypes

import concourse.bass as bass
import concourse.mybir as mybir
from concourse import bacc
import concourse.tile as tile

F32 = mybir.dt.float32
F32R = mybir.dt.float32r
BF16 = mybir.dt.bfloat16
FP8 = mybir.dt.float8e4
BF16_NP = ml_dtypes.bfloat16
FP8_NP = mybir.dt.np(FP8)
# q/k weights are scaled by 16 on the host so fp8e4 stays out of the
# subnormal range; q.k scores come out 256x large, compensated in the
# exp's scale argument
W8_SCALE = 16.0

N = 4
L = 2048
D = 1024
H = 16
P = 64
NCORES = 8
FC = D // NCORES       # features per core = 128
H2 = 2                 # heads per core
EPS = 1e-5
SCALE = 1.0 / math.sqrt(D)
NL = N * L             # 8192 rows in the global batch norm

# software-pipeline stage lags (consumers trail producers so engine wait
# queues never backpressure the sequencers)
EXP_LAG = 1
MASK_LAG = 2
AV_LAG = 3
DEBUG_NOLAG = False

_cached = {}


def r(ap):
    return ap.bitcast(F32R)


def build_program(l=L):
    """Build the SPMD Bass program (identical on all 8 cores)."""
    nc = bacc.Bacc("TRN2", target_bir_lowering=False, debug=False,
                   num_devices=NCORES)

    ic_n = l // 512        # 512-row query chunks
    nlc = N * l            # rows per core (all batches)

    xq8_nd = nc.dram_tensor("xq8_nd", [D, nlc], FP8,
                            kind="ExternalInput").ap()
    xk8_nd = nc.dram_tensor("xk8_nd", [D, nlc], FP8,
                            kind="ExternalInput").ap()
    xk_nd = nc.dram_tensor("xk_nd", [D, nlc], BF16, kind="ExternalInput").ap()
    wq8 = nc.dram_tensor("wq8", [D, FC], FP8, kind="ExternalInput").ap()
    wk8 = nc.dram_tensor("wk8", [D, FC], FP8, kind="ExternalInput").ap()
    wvt = nc.dram_tensor("wvt", [D, FC], BF16, kind="ExternalInput").ap()
    q_res = nc.dram_tensor("q_res", [nlc, FC], BF16, kind="ExternalInput").ap()
    gamma = nc.dram_tensor("gamma", [1, FC], F32, kind="ExternalInput").ap()
    beta = nc.dram_tensor("beta", [1, FC], F32, kind="ExternalInput").ap()
    out_s = nc.dram_tensor("out_s", [nlc, FC], BF16,
                           kind="ExternalOutput").ap()

    jblocks = l // 128     # 16 key blocks per batch
    nls = nlc // 128       # 64 ls blocks of res

    from contextlib import ExitStack
    with ExitStack() as stack:
        tc = stack.enter_context(tile.TileContext(nc))
        pool = {}
        for nm, bufs, space in (
                ("consts", 1, None), ("persist", 1, None), ("wt", 1, None),
                ("qtp", 2, None), ("xq", 2, None), ("xk", 2, None),
                ("xk8", 2, None),
                ("at2", 3, None), ("qin", 2, None), ("sq", 2, None),
                ("outp", 3, None), ("bnp", 1, None), ("small", 6, None),
                ("st2", 2, "PSUM"), ("av", 1, "PSUM"), ("pj", 1, "PSUM"),
                ("stat", 1, "PSUM")):
            kw = {"name": nm, "bufs": bufs}
            if space:
                kw["space"] = space
            pool[nm] = stack.enter_context(tc.tile_pool(**kw))
        consts, persist, wtp = pool["consts"], pool["persist"], pool["wt"]
        qtp, xqp, xkp = pool["qtp"], pool["xq"], pool["xk"]
        xk8p = pool["xk8"]
        at2p, qinp, sqp = pool["at2"], pool["qin"], pool["sq"]
        outp, bnp, smallp = pool["outp"], pool["bnp"], pool["small"]
        st2p, avp, pjp, statp = (pool["st2"], pool["av"], pool["pj"],
                                 pool["stat"])

        # ---------------- first activation chunk DMAs (critical path) ----
        def x_src(dram, n, ic):
            return bass.AP(
                tensor=dram.tensor,
                offset=dram.offset + n * l + ic * 512,
                ap=[[nlc, 128], [128 * nlc, 8], [1, 512]])

        def load_x(pool, dram, n, ic):
            t = pool.tile([128, 8 * 512], BF16, tag="x", name="xt")
            nc.sync.dma_start(
                t.rearrange("p (dc x) -> p dc x", dc=8), x_src(dram, n, ic))
            return t

        def x8_src(dram, n, ic):
            # d = s*256 + t*128 + p (fp8 DoubleRow pair layout)
            return bass.AP(
                tensor=dram.tensor,
                offset=dram.offset + n * l + ic * 512,
                ap=[[nlc, 128], [256 * nlc, 4], [128 * nlc, 2], [1, 512]])

        def load_x8(pool, dram, n, ic):
            t = pool.tile([128, 4 * 2 * 512], FP8, tag="x8", name="x8t")
            nc.sync.dma_start(
                t.rearrange("p (s t x) -> p s t x", s=4, t=2),
                x8_src(dram, n, ic))
            return t

        # first q-projection chain needs wq then xq: emit those two DMAs
        # first so PE can start as early as possible
        wts = {}

        def load_w8(wname, wdram):
            t = wtp.tile([128, 4 * 2 * FC], FP8, tag=wname, name=wname)
            nc.sync.dma_start(
                t.rearrange("p (s t f) -> p s t f", s=4, t=2),
                bass.AP(tensor=wdram.tensor, offset=wdram.offset,
                        ap=[[FC, 128], [256 * FC, 4], [128 * FC, 2],
                            [1, FC]]))
            wts[wname] = t

        load_w8("wq", wq8)
        xq_t = load_x8(xqp, xq8_nd, 0, 0)
        load_w8("wk", wk8)
        xk8_t = load_x8(xk8p, xk8_nd, 0, 0)
        xk_t = load_x(xkp, xk_nd, 0, 0)
        wvt_sb = wtp.tile([128, 8 * FC], BF16, tag="wv", name="wv")
        nc.sync.dma_start(
            wvt_sb.rearrange("p (dc f) -> p dc f", dc=8),
            bass.AP(tensor=wvt.tensor, offset=wvt.offset,
                    ap=[[FC, 128], [128 * FC, 8], [1, FC]]))
        wts["wv"] = wvt_sb

        # ---------------- constants -------------------------------------
        ones_col = consts.tile([128, 1], BF16)
        nc.vector.memset(ones_col, 1.0)
        eps_sb = consts.tile([128, 1], F32)
        nc.vector.memset(eps_sb, EPS)
        gamma_sb = consts.tile([1, FC], F32)
        nc.sync.dma_start(gamma_sb, gamma)
        beta_sb = consts.tile([1, FC], F32)
        nc.sync.dma_start(beta_sb, beta)
        # lower-triangular (j <= i) mask in [j-part, i-free] layout
        tm_f = consts.tile([128, 128], F32)
        nc.vector.memset(tm_f, 1.0)
        nc.gpsimd.affine_select(
            out=tm_f, in_=tm_f,
            compare_op=mybir.AluOpType.is_ge, fill=0.0, base=0,
            pattern=[[1, 128]], channel_multiplier=-1)
        trimask = consts.tile([128, 128], BF16)
        nc.vector.tensor_copy(trimask, tm_f)
        # zero operands for the PSUM-bank-clearing matmuls (PSUM
        # start_tensor_calc marks a whole 2KB zero-region, so banks shared
        # by several accumulation groups must be cleared by one explicit
        # full-tile start matmul instead of per-group start flags)
        zlhs = consts.tile([128, 128], BF16)
        nc.vector.memset(zlhs, 0.0)
        zrhs = consts.tile([128, 260], BF16)
        nc.vector.memset(zrhs, 0.0)

        # ---------------- persistent SBUF -------------------------------
        # kt_sb: [feature(h*64+p), (n, j)] bf16
        kt_sb = persist.tile([128, N * l], BF16, tag="kt")
        # v_sb: [j-in-block, (n, jc, h, 65)] bf16; col 64 of each 65-group
        # is the baked ones column (softmax denominator trick)
        v_sb = persist.tile([128, N * jblocks * H2 * 65], BF16, tag="v")
        v3 = v_sb.rearrange("p (g x) -> p g x", x=65)
        nc.gpsimd.memset(v3[:, :, 64:65], 1.0)
        # res_sb: [l-in-block, (ls, f)] bf16, ls = n*16 + ic*4 + S
        res_sb = persist.tile([128, nls * FC], BF16, tag="res")

        # partition 0 row: sums; partition 64 row: sums of squares
        # (matmul outputs must start at partition 0, 32, or 64)
        stat = statp.tile([65, 512], F32, tag="stat")

        # ------------------------------------------------------------------
        # projection task machinery (filler micro-ops paced into B loops)
        # ------------------------------------------------------------------
        # rotating PSUM allocators: the interleaved fillers use the single
        # pj bank; the up-front A(0) block also rotates through the two
        # (then idle) score banks so chains overlap their copy-out
        def alloc_pj():
            return pjp.tile([128, 512], F32, tag="pj", name="pj")

        def alloc_st2_slot():
            return st2p.tile([128, 1024], F32, tag="st2", name="st2")[:, 0:512]

        rot = {"i": 0}

        def alloc_rotating():
            rot["i"] += 1
            return alloc_pj() if rot["i"] % 3 == 0 else alloc_st2_slot()

        chain_alloc = {"fn": alloc_pj}

        def qk_chain(side, n, ic, qt_tile, get_xt):
            """q/k projection via fp8 DoubleRow: 4 contraction-256 steps."""
            w_use = wts["wq"] if side == "q" else wts["wk"]
            w4 = w_use.rearrange("p (s t f) -> p s t f", s=4, t=2)
            pj = {}
            alloc_fn = chain_alloc["fn"]

            def alloc():
                pj["t"] = alloc_fn()

            def mm(s0):
                x4 = get_xt().rearrange("p (s t x) -> p s t x", s=4, t=2)
                for s in (s0, s0 + 1):
                    nc.tensor.matmul(
                        pj["t"], w4[:, s], x4[:, s],
                        start=(s == 0), stop=(s == 3),
                        perf_mode=mybir.MatmulPerfMode.DoubleRow)

            def copy():
                if side == "q":
                    nc.vector.tensor_copy(qt_tile[:, n * 512:(n + 1) * 512],
                                          pj["t"])
                else:
                    nc.vector.tensor_copy(
                        kt_sb[:, n * l + ic * 512:n * l + ic * 512 + 512],
                        pj["t"])

            ops = [alloc]
            for s0 in range(0, 4, 2):
                ops.append(lambda s=s0: mm(s))
            ops.append(copy)
            return ops

        def wts_slice(w, dc):
            return w[:, dc * FC:(dc + 1) * FC]

        def v_chain(n, jsub, ic, get_xt):
            pj = {}
            alloc_fn = chain_alloc["fn"]

            def alloc():
                pj["t"] = alloc_fn()

            def mm(dc0):
                xt = get_xt()
                for dc in (dc0, dc0 + 1):
                    nc.tensor.matmul(
                        pj["t"][:, 0:128],
                        xt[:, dc * 512 + jsub * 128:dc * 512 + jsub * 128 + 128],
                        wts_slice(wts["wv"], dc),
                        start=(dc == 0), stop=(dc == 7))

            def copy():
                jc = ic * 4 + jsub
                base = (n * jblocks + jc) * H2 * 65
                dst = v_sb[:, base:base + 130].rearrange(
                    "p (h x) -> p h x", h=2)[:, :, 0:64]
                src = pj["t"][:, 0:128].rearrange("p (h x) -> p h x", h=2)
                nc.vector.tensor_copy(dst, src)

            ops = [alloc]
            for dc0 in range(0, 8, 2):
                ops.append(lambda d=dc0: mm(d))
            ops.append(copy)
            return ops

        def build_chunk_groups(ic, qt_tile, first_x, rotate_first=False):
            """Return per-batch lists of micro-op closures for A(ic)."""
            groups = []
            xq_cur = {0: first_x[0]}
            xk8_cur = {0: first_x[1]}
            xk_cur = {0: first_x[2]}
            for n in range(N):
                chain_alloc["fn"] = (alloc_rotating if rotate_first and n == 0
                                     else alloc_pj)
                ops = []
                get_xq = lambda nn=n: xq_cur[nn]
                get_xk8 = lambda nn=n: xk8_cur[nn]
                get_xk = lambda nn=n: xk_cur[nn]
                # prefetch next batch's activations
                if n + 1 < N:
                    def pre(nn=n + 1):
                        xq_cur[nn] = load_x8(xqp, xq8_nd, nn, ic)
                        xk8_cur[nn] = load_x8(xk8p, xk8_nd, nn, ic)
                        xk_cur[nn] = load_x(xkp, xk_nd, nn, ic)
                    ops.append(pre)
                ops += qk_chain("q", n, ic, qt_tile, get_xq)
                ops += qk_chain("k", n, ic, None, get_xk8)
                for jsub in range(4):
                    ops += v_chain(n, jsub, ic, get_xk)
                groups.append(ops)
            return groups

        # ------------------------------------------------------------------
        # A(0): only batch 0's projections run up front; batches 1-3 are
        # deadline-paced into B(0)'s iterations (batch n is needed at
        # B(0) iteration 4n) so the scalar engine starts exp'ing early.
        # ------------------------------------------------------------------
        qt_next = qtp.tile([128, N * 512], BF16, tag="qt", name="qt")
        groups0 = build_chunk_groups(0, qt_next, (xq_t, xk8_t, xk_t),
                                     rotate_first=True)
        for op in groups0[0]:
            op()

        # ------------------------------------------------------------------
        # main loop: one software-pipelined stream over (ic, n, jc).
        # Stage schedule at step t: scores(t), exp(t-1), mask(t-2), AV(t-3)
        # so every instruction's inputs are ready when the engine decodes
        # it (the 4-deep per-engine wait queues otherwise backpressure the
        # sequencers). A(ic+1) projection micro-ops are paced in as filler.
        # ------------------------------------------------------------------
        specs = []
        for ic in range(ic_n):
            for n in range(N):
                for jc in range(4 * ic + 4):
                    specs.append((ic, n, jc))
        nspec = len(specs)
        qt_tiles = {0: qt_next}
        st2_of, at2_of, avs_of, qres_of = {}, {}, {}, {}
        filler = []
        quota = {}

        def emit_filler(k):
            for _ in range(k):
                if filler:
                    filler.pop(0)()

        def stage_scores(idx):
            ic, n, jc = specs[idx]
            st2 = st2p.tile([128, 1024], F32, tag="st2", name="st2")
            st2_of[idx] = st2
            qt_cur = qt_tiles[ic]
            for h in range(H2):
                nc.tensor.matmul(
                    st2[:, h * 512:(h + 1) * 512],
                    kt_sb[h * 64:(h + 1) * 64,
                          n * l + jc * 128:n * l + jc * 128 + 128],
                    qt_cur[h * 64:(h + 1) * 64, n * 512:(n + 1) * 512],
                    start=True, stop=True)

        def stage_exp(idx):
            ic, n, jc = specs[idx]
            rr = jc - 4 * ic
            st2 = st2_of.pop(idx)
            at2 = at2p.tile([128, 1024], BF16, tag="at2", name="at2")
            at2_of[idx] = at2
            if rr <= 0:
                nc.scalar.activation(at2, st2,
                                     mybir.ActivationFunctionType.Exp,
                                     scale=SCALE / (W8_SCALE * W8_SCALE))
            else:
                for h in range(H2):
                    nc.scalar.activation(
                        at2[:, h * 512 + rr * 128:(h + 1) * 512],
                        st2[:, h * 512 + rr * 128:(h + 1) * 512],
                        mybir.ActivationFunctionType.Exp,
                        scale=SCALE / (W8_SCALE * W8_SCALE))

        def stage_mask(idx):
            ic, n, jc = specs[idx]
            rr = jc - 4 * ic
            if rr < 0:
                return
            at2 = at2_of[idx]
            for h in range(H2):
                sl = slice(h * 512 + rr * 128, h * 512 + rr * 128 + 128)
                nc.vector.tensor_mul(at2[:, sl], at2[:, sl], trimask)

        def stage_av(idx):
            ic, n, jc = specs[idx]
            rr = jc - 4 * ic
            at2 = at2_of.pop(idx)
            if jc == 0:
                avs_of[(ic, n)] = [avp.tile([128, 260], F32, tag=f"av{h}",
                                            name=f"av{h}")
                                   for h in range(H2)]
                for h in range(H2):
                    # clear the whole accumulator bank exactly once
                    nc.tensor.matmul(avs_of[(ic, n)][h], zlhs, zrhs,
                                     start=True, stop=True,
                                     skip_group_check=True)
            avs = avs_of[(ic, n)]
            vbase = (n * jblocks + jc) * H2 * 65
            for h in range(H2):
                for S in range(4):
                    if rr > S:
                        continue
                    nc.tensor.matmul(
                        avs[h][:, S * 65:(S + 1) * 65],
                        at2[:, h * 512 + S * 128:h * 512 + S * 128 + 128],
                        v_sb[:, vbase + h * 65:vbase + h * 65 + 65],
                        start=False, stop=(rr == S),
                        skip_group_check=True)
            if jc == 4 * ic + 3:
                enqueue_drain(ic, n)

        # drains and stats run as small deferred pieces, one per iteration,
        # so their engine dependencies are satisfied before dispatch and
        # they never block the in-order PE/DVE queues.
        drain_pending = []

        def enqueue_drain(ic, n):
            avs = avs_of.pop((ic, n))
            qres_t = qres_of.pop((ic, n))
            base512 = (n * 16 + ic * 4) * FC

            def drain_head(h):
                av3 = avs[h].rearrange("p (s x) -> p s x", x=65)
                rec = smallp.tile([128, 4], F32, tag="rec", name="rec")
                nc.vector.reciprocal(rec, av3[:, :, 64])
                for S in range(4):
                    nc.vector.scalar_tensor_tensor(
                        out=res_sb[:, base512 + S * FC + h * 64:
                                   base512 + S * FC + h * 64 + 64],
                        in0=avs[h][:, S * 65:S * 65 + 64],
                        scalar=rec[:, S:S + 1],
                        in1=qres_t[:, S * FC + h * 64:S * FC + h * 64 + 64],
                        op0=mybir.AluOpType.mult,
                        op1=mybir.AluOpType.add)

            def drain_sq():
                res_block = res_sb[:, base512:base512 + 512]
                sqt = sqp.tile([128, 512], BF16, tag="sq", name="sqt")
                nc.vector.tensor_mul(sqt, res_block, res_block)
                stats_bufs[(ic, n)] = (res_block, sqt)

            def drain_stats():
                res_block, sqt = stats_bufs.pop((ic, n))
                first = (n == 0 and ic == 0)
                last = (n == N - 1 and ic == ic_n - 1)
                nc.tensor.matmul(stat[0:1, :], ones_col, res_block,
                                 start=first, stop=last,
                                 skip_group_check=True)
                nc.tensor.matmul(stat[64:65, :], ones_col, sqt,
                                 start=first, stop=last,
                                 skip_group_check=True)

            if DEBUG_NOLAG:
                drain_head(0)
                drain_head(1)
                drain_sq()
                drain_stats()
            else:
                drain_pending.extend(
                    [lambda: drain_head(0), None,
                     lambda: (drain_head(1), drain_sq()), None, drain_stats])

        stats_bufs = {}

        def step_drain(flush=False):
            while drain_pending:
                op = drain_pending.pop(0)
                if op is None:
                    if flush:
                        continue
                    return
                op()

        # chunk 0's remaining projection groups (batches 1-3) are due just
        # before B(0) reaches that batch: group n spread over iters
        # [4(n-1), 4n)
        for n in range(1, N):
            ops = groups0[n]
            no = len(ops)
            for t in range(4):
                quota[4 * (n - 1) + t] = (quota.get(4 * (n - 1) + t, 0)
                                          + ((t + 1) * no) // 4
                                          - (t * no) // 4)
        filler = groups0[1] + groups0[2] + groups0[3]

        for idx in range(nspec + max(EXP_LAG, MASK_LAG, AV_LAG)):
            if idx < nspec:
                ic, n, jc = specs[idx]
                if jc == 0 and n == 0 and ic + 1 < ic_n:
                    # build next chunk's projection fillers, paced over
                    # this chunk's iterations (merged with any deadline
                    # quotas already scheduled for these slots)
                    if ic > 0:
                        emit_filler(len(filler))
                    qt_tiles[ic + 1] = qtp.tile([128, N * 512], BF16,
                                                tag="qt", name="qt")
                    nxq = load_x8(xqp, xq8_nd, 0, ic + 1)
                    nxk8 = load_x8(xk8p, xk8_nd, 0, ic + 1)
                    nxk = load_x(xkp, xk_nd, 0, ic + 1)
                    groups = build_chunk_groups(ic + 1, qt_tiles[ic + 1],
                                                (nxq, nxk8, nxk))
                    new_ops = [op for g in groups for op in g]
                    iters = N * (4 * ic + 4)
                    nf = len(new_ops)
                    for t in range(iters):
                        quota[idx + t] = (quota.get(idx + t, 0)
                                          + ((t + 1) * nf) // iters
                                          - (t * nf) // iters)
                    filler.extend(new_ops)
                if jc == 0:
                    qres_t = qinp.tile([128, 4 * FC], BF16, tag="qres",
                                       name="qres")
                    nc.sync.dma_start(
                        qres_t.rearrange("p (s f) -> p s f", s=4),
                        bass.AP(tensor=q_res.tensor,
                                offset=q_res.offset + (n * l + ic * 512) * FC,
                                ap=[[FC, 128], [128 * FC, 4], [1, FC]]))
                    qres_of[(ic, n)] = qres_t
                stage_scores(idx)
            if idx - EXP_LAG >= 0 and idx - EXP_LAG < nspec:
                stage_exp(idx - EXP_LAG)
            if idx - MASK_LAG >= 0 and idx - MASK_LAG < nspec:
                stage_mask(idx - MASK_LAG)
            step_drain()
            if idx - AV_LAG >= 0 and idx - AV_LAG < nspec:
                stage_av(idx - AV_LAG)
            emit_filler(quota.get(idx, 0))
            if idx == nspec - 1:
                emit_filler(len(filler))
        step_drain(flush=True)

        # ------------------------------------------------------------------
        # batch-norm: fold partial sums, compute gamma', beta', apply
        # ------------------------------------------------------------------
        # fold the 4 ls-group partials to [1, FC] on partition 0
        sum_r = bnp.tile([1, 512], F32, tag="sumr", name="sumr")
        nc.vector.tensor_copy(sum_r, stat[0:1, :])
        sq_r = bnp.tile([1, 512], F32, tag="sqr", name="sqr")
        nc.vector.tensor_copy(sq_r, stat[64:65, :])
        sA = bnp.tile([1, FC], F32, tag="sA", name="sA")
        nc.vector.tensor_add(sA, sum_r[:, 0:FC], sum_r[:, FC:2 * FC])
        sB = bnp.tile([1, FC], F32, tag="sB", name="sB")
        nc.vector.tensor_add(sB, sum_r[:, 2 * FC:3 * FC],
                             sum_r[:, 3 * FC:4 * FC])
        sumf = bnp.tile([1, FC], F32, tag="sumf", name="sumf")
        nc.vector.tensor_add(sumf, sA, sB)
        qA = bnp.tile([1, FC], F32, tag="qA", name="qA")
        nc.vector.tensor_add(qA, sq_r[:, 0:FC], sq_r[:, FC:2 * FC])
        qB = bnp.tile([1, FC], F32, tag="qB", name="qB")
        nc.vector.tensor_add(qB, sq_r[:, 2 * FC:3 * FC],
                             sq_r[:, 3 * FC:4 * FC])
        sqf = bnp.tile([1, FC], F32, tag="sqf", name="sqf")
        nc.vector.tensor_add(sqf, qA, qB)

        inv = 1.0 / NL
        mean = bnp.tile([1, FC], F32, tag="mean", name="mean")
        nc.vector.tensor_scalar_mul(mean, sumf, inv)
        musq = bnp.tile([1, FC], F32, tag="musq", name="musq")   # mean^2
        nc.vector.tensor_mul(musq, mean, mean)
        var = bnp.tile([1, FC], F32, tag="var", name="var")
        nc.vector.scalar_tensor_tensor(
            out=var, in0=sqf, scalar=inv, in1=musq,
            op0=mybir.AluOpType.mult, op1=mybir.AluOpType.subtract)
        std = bnp.tile([1, FC], F32, tag="std", name="std")
        nc.scalar.activation(std, var, mybir.ActivationFunctionType.Sqrt,
                             bias=eps_sb[0:1, :])
        rstd = bnp.tile([1, FC], F32, tag="rstd", name="rstd")
        nc.vector.reciprocal(rstd, std)
        gp = bnp.tile([1, FC], F32, tag="gp", name="gp")
        nc.vector.tensor_mul(gp, gamma_sb, rstd)
        mgp = bnp.tile([1, FC], F32, tag="mgp", name="mgp")
        nc.vector.tensor_mul(mgp, mean, gp)
        bp = bnp.tile([1, FC], F32, tag="bp", name="bp")
        nc.vector.tensor_sub(bp, beta_sb, mgp)
        gp16 = bnp.tile([1, FC], BF16, tag="gp16", name="gp16")
        nc.vector.tensor_copy(gp16, gp)
        bp16 = bnp.tile([1, FC], BF16, tag="bp16", name="bp16")
        nc.vector.tensor_copy(bp16, bp)

        gbc = bnp.tile([128, FC], BF16, tag="gbc", name="gbc")
        nc.gpsimd.partition_broadcast(gbc, gp16)
        bbc = bnp.tile([128, FC], BF16, tag="bbc", name="bbc")
        nc.gpsimd.partition_broadcast(bbc, bp16)

        def rep4(t):
            return bass.AP(tensor=t.tensor, offset=t.offset,
                           ap=[[t.ap[0][0], 128], [0, 4], [1, FC]])

        gbc4 = bnp.tile([128, 512], BF16, tag="gbc4", name="gbc4")
        nc.vector.tensor_copy(gbc4, rep4(gbc))
        bbc4 = bnp.tile([128, 512], BF16, tag="bbc4", name="bbc4")
        nc.vector.tensor_copy(bbc4, rep4(bbc))

        for n in range(N):
            for ic in range(ic_n):
                base512 = (n * 16 + ic * 4) * FC
                t1 = outp.tile([128, 512], BF16, tag="t1", name="t1")
                nc.vector.tensor_mul(t1, res_sb[:, base512:base512 + 512],
                                     gbc4)
                t2 = outp.tile([128, 512], BF16, tag="t2", name="t2")
                nc.vector.tensor_add(t2, t1, bbc4)
                nc.sync.dma_start(
                    bass.AP(tensor=out_s.tensor,
                            offset=out_s.offset + (n * l + ic * 512) * FC,
                            ap=[[FC, 128], [128 * FC, 4], [1, FC]]),
                    t2.rearrange("p (s f) -> p s f", s=4))

    nc.compile()
    return nc


def get_runner(nc):
    """Build (once) a cached jitted SPMD executor for the Bass program."""
    if "runner" in _cached:
        return _cached["runner"]

    import jax
    from jax.experimental.shard_map import shard_map
    from jax.sharding import Mesh, PartitionSpec
    from concourse import bass2jax

    bass2jax.install_neuronx_cc_hook()

    partition_name = (nc.partition_id_tensor.name
                      if nc.partition_id_tensor else None)
    in_names, out_names, out_avals, zero_outs = [], [], [], []
    for alloc in nc.m.functions[0].allocations:
        if not isinstance(alloc, mybir.MemoryLocationSet):
            continue
        name = alloc.memorylocations[0].name
        if alloc.kind == "ExternalInput":
            if name != partition_name:
                in_names.append(name)
        elif alloc.kind == "ExternalOutput":
            shape = tuple(alloc.tensor_shape)
            dtype = mybir.dt.np(alloc.dtype)
            out_names.append(name)
            out_avals.append(jax.core.ShapedArray(shape, dtype))
            zero_outs.append(np.zeros(shape, dtype))
    n_params = len(in_names)
    n_outs = len(out_avals)
    all_names = in_names + out_names
    if partition_name is not None:
        all_names = all_names + [partition_name]

    def _body(*args):
        operands = list(args)
        if partition_name is not None:
            operands.append(bass2jax.partition_id_tensor())
        outs = bass2jax._bass_exec_p.bind(
            *operands,
            out_avals=tuple(out_avals),
            in_names=tuple(all_names),
            out_names=tuple(out_names),
            lowering_input_output_aliases=(),
            sim_require_finite=True,
            sim_require_nnan=True,
            nc=nc,
        )
        return tuple(outs)

    devices = jax.devices()[:NCORES]
    mesh = Mesh(np.asarray(devices), ("core",))
    in_specs = (PartitionSpec("core"),) * (n_params + n_outs)
    out_specs = (PartitionSpec("core"),) * n_outs
    donate = tuple(range(n_params, n_params + n_outs))
    sharded = jax.jit(
        shard_map(_body, mesh=mesh, in_specs=in_specs, out_specs=out_specs,
                  check_rep=False),
        donate_argnums=donate, keep_unused=True)

    def run_np(in_maps):
        concat_in = [
            np.concatenate([np.asarray(in_maps[c][nm]) for c in range(NCORES)],
                           axis=0)
            for nm in in_names]
        concat_zeros = [np.zeros((NCORES * z.shape[0], *z.shape[1:]), z.dtype)
                        for z in zero_outs]
        out_arrs = sharded(*concat_in, *concat_zeros)
        return [
            {nm: np.asarray(out_arrs[i]).reshape(
                NCORES, *out_avals[i].shape)[c]
             for i, nm in enumerate(out_names)}
            for c in range(NCORES)]

    _cached["runner"] = (run_np, sharded, in_names, out_names, out_avals,
                         zero_outs, mesh)
    return _cached["runner"]


def make_in_maps(inputs, l):
    query = np.asarray(inputs["query"], dtype=np.float32)
    key = np.asarray(inputs["key"], dtype=np.float32)
    Wq = np.asarray(inputs["Wq"], dtype=np.float32)
    Wk = np.asarray(inputs["Wk"], dtype=np.float32)
    Wv = np.asarray(inputs["Wv"], dtype=np.float32)
    gamma = np.asarray(inputs["gamma"], dtype=np.float32)
    beta = np.asarray(inputs["beta"], dtype=np.float32)

    n = query.shape[0]
    qf = query.reshape(n * l, D)
    kf = key.reshape(n * l, D)
    xq8 = np.ascontiguousarray(qf.T.astype(FP8_NP))
    xk8 = np.ascontiguousarray(kf.T.astype(FP8_NP))
    xk = np.ascontiguousarray(kf.T.astype(BF16_NP))

    in_maps = []
    for c in range(NCORES):
        sl = slice(c * FC, (c + 1) * FC)
        in_maps.append({
            "xq8_nd": xq8,
            "xk8_nd": xk8,
            "xk_nd": xk,
            "wq8": np.ascontiguousarray(
                (Wq[sl].T * W8_SCALE).astype(FP8_NP)),
            "wk8": np.ascontiguousarray(
                (Wk[sl].T * W8_SCALE).astype(FP8_NP)),
            "wvt": np.ascontiguousarray(Wv[sl].T.astype(BF16_NP)),
            "q_res": np.ascontiguousarray(qf[:, sl].astype(BF16_NP)),
            "gamma": np.ascontiguousarray(gamma[sl].reshape(1, FC)),
            "beta": np.ascontiguousarray(beta[sl].reshape(1, FC)),
        })
    return in_maps


def kernel(**inputs):
    l = np.asarray(inputs["query"]).shape[1]
    if "nc" not in _cached or _cached.get("l") != l:
        _cached["nc"] = build_program(l)
        _cached["l"] = l
    nc = _cached["nc"]

    in_maps = make_in_maps(inputs, l)
    run_np = get_runner(nc)[0]
    results = run_np(in_maps)

    n = np.asarray(inputs["query"]).shape[0]
    out = np.zeros((n, l, D), dtype=np.float32)
    for c in range(NCORES):
        sl = slice(c * FC, (c + 1) * FC)
        out[:, :, sl] = results[c]["out_s"].reshape(n, l, FC).astype(
            np.float32)
    return out


# revision 58
# speedup vs baseline: 1.2483x; 1.0922x over previous
"""Trainium2 Bass kernel for MultiHeadAttention + residual + BatchNorm.

Model (reference):
  q = query @ Wq.T ; k = key @ Wk.T ; v = key @ Wv.T    (per-head split)
  score = q k^T / sqrt(D), causal mask, softmax over keys
  res   = (attn @ v) + query
  out   = batchnorm(res over all (N*L) rows, per feature) * gamma + beta

Sharding over 8 cores: FEATURE sharding. Core c owns heads {2c, 2c+1}
(features [128c, 128c+128)) for ALL batches. BatchNorm statistics are
then core-local (sums over all N*L rows of the core's own features), so
no collective is needed at all.

All matmul operands are bf16 (PSUM accumulation stays fp32); the
residual add, batch-norm statistics and outputs are fp32.

Per 512-row query chunk ic and batch n the attention inner loop walks
key blocks jc (128 keys each, causal): PE computes both heads' scores
into one PSUM tile [128 j, 1024 (h,i)], ScalarE applies a single merged
exp -> bf16, DVE masks the diagonal block with a triangular multiply,
and PE accumulates the *flipped* attention-V product out[i, p] with a
ones-column appended to V so softmax denominators fall out of the same
matmuls. Projections for chunk ic+1 are emitted as small PE micro-ops
paced between attention iterations so the scalar engine never starves.
"""

import math
import sys

sys.path.insert(0, "/opt/trn_rl_repo")

import numpy as np
import ml_dtypes

import concourse.bass as bass
import concourse.mybir as mybir
from concourse import bacc
import concourse.tile as tile

F32 = mybir.dt.float32
F32R = mybir.dt.float32r
BF16 = mybir.dt.bfloat16
FP8 = mybir.dt.float8e4
BF16_NP = ml_dtypes.bfloat16
FP8_NP = mybir.dt.np(FP8)
# q/k weights are scaled by 16 on the host so fp8e4 stays out of the
# subnormal range; q.k scores come out 256x large, compensated in the
# exp's scale argument
W8_SCALE = 16.0

N = 4
L = 2048
D = 1024
H = 16
P = 64
NCORES = 8
FC = D // NCORES       # features per core = 128
H2 = 2                 # heads per core
EPS = 1e-5
SCALE = 1.0 / math.sqrt(D)
NL = N * L             # 8192 rows in the global batch norm

# software-pipeline stage lags (consumers trail producers so engine wait
# queues never backpressure the sequencers)
EXP_LAG = 1
MASK_LAG = 2
AV_LAG = 3
DEBUG_NOLAG = False

_cached = {}


def r(ap):
    return ap.bitcast(F32R)


def build_program(l=L):
    """Build the SPMD Bass program (identical on all 8 cores)."""
    nc = bacc.Bacc("TRN2", target_bir_lowering=False, debug=False,
                   num_devices=NCORES)

    ic_n = l // 512        # 512-row query chunks
    nlc = N * l            # rows per core (all batches)

    xq8_nd = nc.dram_tensor("xq8_nd", [D, nlc], FP8,
                            kind="ExternalInput").ap()
    xk8_nd = nc.dram_tensor("xk8_nd", [D, nlc], FP8,
                            kind="ExternalInput").ap()
    wq8 = nc.dram_tensor("wq8", [D, FC], FP8, kind="ExternalInput").ap()
    wk8 = nc.dram_tensor("wk8", [D, FC], FP8, kind="ExternalInput").ap()
    wvt = nc.dram_tensor("wvt", [D, FC], FP8, kind="ExternalInput").ap()
    q_res = nc.dram_tensor("q_res", [nlc, FC], BF16, kind="ExternalInput").ap()
    gamma = nc.dram_tensor("gamma", [1, FC], F32, kind="ExternalInput").ap()
    beta = nc.dram_tensor("beta", [1, FC], F32, kind="ExternalInput").ap()
    out_s = nc.dram_tensor("out_s", [nlc, FC], BF16,
                           kind="ExternalOutput").ap()

    jblocks = l // 128     # 16 key blocks per batch
    nls = nlc // 128       # 64 ls blocks of res

    from contextlib import ExitStack
    with ExitStack() as stack:
        tc = stack.enter_context(tile.TileContext(nc))
        pool = {}
        for nm, bufs, space in (
                ("consts", 1, None), ("persist", 1, None), ("wt", 1, None),
                ("qtp", 2, None), ("xq", 2, None), ("xk", 2, None),
                ("xk8", 2, None),
                ("at2", 3, None), ("qin", 2, None), ("sq", 2, None),
                ("outp", 3, None), ("bnp", 1, None), ("small", 6, None),
                ("st2", 2, "PSUM"), ("av", 1, "PSUM"), ("pj", 1, "PSUM"),
                ("stat", 1, "PSUM")):
            kw = {"name": nm, "bufs": bufs}
            if space:
                kw["space"] = space
            pool[nm] = stack.enter_context(tc.tile_pool(**kw))
        consts, persist, wtp = pool["consts"], pool["persist"], pool["wt"]
        qtp, xqp, xkp = pool["qtp"], pool["xq"], pool["xk"]
        xk8p = pool["xk8"]
        at2p, qinp, sqp = pool["at2"], pool["qin"], pool["sq"]
        outp, bnp, smallp = pool["outp"], pool["bnp"], pool["small"]
        st2p, avp, pjp, statp = (pool["st2"], pool["av"], pool["pj"],
                                 pool["stat"])

        # ---------------- first activation chunk DMAs (critical path) ----
        def x_src(dram, n, ic):
            return bass.AP(
                tensor=dram.tensor,
                offset=dram.offset + n * l + ic * 512,
                ap=[[nlc, 128], [128 * nlc, 8], [1, 512]])

        def load_x(pool, dram, n, ic):
            t = pool.tile([128, 8 * 512], BF16, tag="x", name="xt")
            nc.sync.dma_start(
                t.rearrange("p (dc x) -> p dc x", dc=8), x_src(dram, n, ic))
            return t

        def x8_src(dram, n, ic):
            # d = s*256 + t*128 + p (fp8 DoubleRow pair layout)
            return bass.AP(
                tensor=dram.tensor,
                offset=dram.offset + n * l + ic * 512,
                ap=[[nlc, 128], [256 * nlc, 4], [128 * nlc, 2], [1, 512]])

        def load_x8(pool, dram, n, ic):
            t = pool.tile([128, 4 * 2 * 512], FP8, tag="x8", name="x8t")
            nc.sync.dma_start(
                t.rearrange("p (s t x) -> p s t x", s=4, t=2),
                x8_src(dram, n, ic))
            return t

        # first q-projection chain needs wq then xq: emit those two DMAs
        # first so PE can start as early as possible
        wts = {}

        def load_w8(wname, wdram):
            t = wtp.tile([128, 4 * 2 * FC], FP8, tag=wname, name=wname)
            nc.sync.dma_start(
                t.rearrange("p (s t f) -> p s t f", s=4, t=2),
                bass.AP(tensor=wdram.tensor, offset=wdram.offset,
                        ap=[[FC, 128], [256 * FC, 4], [128 * FC, 2],
                            [1, FC]]))
            wts[wname] = t

        load_w8("wq", wq8)
        xq_t = load_x8(xqp, xq8_nd, 0, 0)
        load_w8("wk", wk8)
        xk8_t = load_x8(xk8p, xk8_nd, 0, 0)
        load_w8("wv", wvt)

        # ---------------- constants -------------------------------------
        ones_col = consts.tile([128, 1], BF16)
        nc.vector.memset(ones_col, 1.0)
        eps_sb = consts.tile([128, 1], F32)
        nc.vector.memset(eps_sb, EPS)
        gamma_sb = consts.tile([1, FC], F32)
        nc.sync.dma_start(gamma_sb, gamma)
        beta_sb = consts.tile([1, FC], F32)
        nc.sync.dma_start(beta_sb, beta)
        # lower-triangular (j <= i) mask in [j-part, i-free] layout
        tm_f = consts.tile([128, 128], F32)
        nc.vector.memset(tm_f, 1.0)
        nc.gpsimd.affine_select(
            out=tm_f, in_=tm_f,
            compare_op=mybir.AluOpType.is_ge, fill=0.0, base=0,
            pattern=[[1, 128]], channel_multiplier=-1)
        trimask = consts.tile([128, 128], BF16)
        nc.vector.tensor_copy(trimask, tm_f)
        # zero operands for the PSUM-bank-clearing matmuls (PSUM
        # start_tensor_calc marks a whole 2KB zero-region, so banks shared
        # by several accumulation groups must be cleared by one explicit
        # full-tile start matmul instead of per-group start flags)
        zlhs = consts.tile([128, 128], BF16)
        nc.vector.memset(zlhs, 0.0)
        zrhs = consts.tile([128, 260], BF16)
        nc.vector.memset(zrhs, 0.0)

        # ---------------- persistent SBUF -------------------------------
        # kt_sb: [feature(h*64+p), (n, j)] bf16
        kt_sb = persist.tile([128, N * l], BF16, tag="kt")
        # v_sb: [j-in-block, (n, jc, h, 65)] bf16; col 64 of each 65-group
        # is the baked ones column (softmax denominator trick)
        v_sb = persist.tile([128, N * jblocks * H2 * 65], BF16, tag="v")
        v3 = v_sb.rearrange("p (g x) -> p g x", x=65)
        nc.gpsimd.memset(v3[:, :, 64:65], 1.0)
        # res_sb: [l-in-block, (ls, f)] bf16, ls = n*16 + ic*4 + S
        res_sb = persist.tile([128, nls * FC], BF16, tag="res")

        # partition 0 row: sums; partition 64 row: sums of squares
        # (matmul outputs must start at partition 0, 32, or 64)
        stat = statp.tile([65, 512], F32, tag="stat")

        # ------------------------------------------------------------------
        # projection task machinery (filler micro-ops paced into B loops)
        # ------------------------------------------------------------------
        # rotating PSUM allocators: the interleaved fillers use the single
        # pj bank; the up-front A(0) block also rotates through the two
        # (then idle) score banks so chains overlap their copy-out
        def alloc_pj():
            return pjp.tile([128, 512], F32, tag="pj", name="pj")

        def alloc_st2_slot():
            return st2p.tile([128, 1024], F32, tag="st2", name="st2")[:, 0:512]

        rot = {"i": 0}

        def alloc_rotating():
            rot["i"] += 1
            return alloc_pj() if rot["i"] % 3 == 0 else alloc_st2_slot()

        chain_alloc = {"fn": alloc_pj}

        def qk_chain(side, n, ic, qt_tile, get_xt):
            """q/k projection via fp8 DoubleRow: 4 contraction-256 steps."""
            w_use = wts["wq"] if side == "q" else wts["wk"]
            w4 = w_use.rearrange("p (s t f) -> p s t f", s=4, t=2)
            pj = {}
            alloc_fn = chain_alloc["fn"]

            def alloc():
                pj["t"] = alloc_fn()

            def mm(s0):
                x4 = get_xt().rearrange("p (s t x) -> p s t x", s=4, t=2)
                for s in (s0, s0 + 1):
                    nc.tensor.matmul(
                        pj["t"], w4[:, s], x4[:, s],
                        start=(s == 0), stop=(s == 3),
                        perf_mode=mybir.MatmulPerfMode.DoubleRow)

            def copy():
                if side == "q":
                    nc.vector.tensor_copy(qt_tile[:, n * 512:(n + 1) * 512],
                                          pj["t"])
                else:
                    nc.vector.tensor_copy(
                        kt_sb[:, n * l + ic * 512:n * l + ic * 512 + 512],
                        pj["t"])

            ops = [alloc]
            for s0 in range(0, 4, 2):
                ops.append(lambda s=s0: mm(s))
            ops.append(copy)
            return ops

        def wts_slice(w, dc):
            return w[:, dc * FC:(dc + 1) * FC]

        def v_chain(n, jsub, ic, get_xt):
            pj = {}
            alloc_fn = chain_alloc["fn"]
            wv4 = wts["wv"].rearrange("p (s t f) -> p s t f", s=4, t=2)

            def alloc():
                pj["t"] = alloc_fn()

            def mm(s0):
                x4 = get_xt().rearrange("p (s t x) -> p s t x", s=4, t=2)
                for s in (s0, s0 + 1):
                    nc.tensor.matmul(
                        pj["t"][:, 0:128],
                        x4[:, s, :, jsub * 128:jsub * 128 + 128],
                        wv4[:, s],
                        start=(s == 0), stop=(s == 3),
                        perf_mode=mybir.MatmulPerfMode.DoubleRow)

            def copy():
                jc = ic * 4 + jsub
                base = (n * jblocks + jc) * H2 * 65
                dst = v_sb[:, base:base + 130].rearrange(
                    "p (h x) -> p h x", h=2)[:, :, 0:64]
                src = pj["t"][:, 0:128].rearrange("p (h x) -> p h x", h=2)
                nc.vector.tensor_copy(dst, src)

            ops = [alloc]
            for s0 in range(0, 4, 2):
                ops.append(lambda s=s0: mm(s))
            ops.append(copy)
            return ops

        def build_chunk_groups(ic, qt_tile, first_x, rotate_first=False):
            """Return per-batch lists of micro-op closures for A(ic)."""
            groups = []
            xq_cur = {0: first_x[0]}
            xk8_cur = {0: first_x[1]}
            for n in range(N):
                chain_alloc["fn"] = (alloc_rotating if rotate_first and n == 0
                                     else alloc_pj)
                ops = []
                get_xq = lambda nn=n: xq_cur[nn]
                get_xk8 = lambda nn=n: xk8_cur[nn]
                # prefetch next batch's activations
                if n + 1 < N:
                    def pre(nn=n + 1):
                        xq_cur[nn] = load_x8(xqp, xq8_nd, nn, ic)
                        xk8_cur[nn] = load_x8(xk8p, xk8_nd, nn, ic)
                    ops.append(pre)
                ops += qk_chain("q", n, ic, qt_tile, get_xq)
                ops += qk_chain("k", n, ic, None, get_xk8)
                for jsub in range(4):
                    ops += v_chain(n, jsub, ic, get_xk8)
                groups.append(ops)
            return groups

        # ------------------------------------------------------------------
        # A(0): only batch 0's projections run up front; batches 1-3 are
        # deadline-paced into B(0)'s iterations (batch n is needed at
        # B(0) iteration 4n) so the scalar engine starts exp'ing early.
        # ------------------------------------------------------------------
        qt_next = qtp.tile([128, N * 512], BF16, tag="qt", name="qt")
        groups0 = build_chunk_groups(0, qt_next, (xq_t, xk8_t),
                                     rotate_first=True)
        for op in groups0[0]:
            op()

        # ------------------------------------------------------------------
        # main loop: one software-pipelined stream over (ic, n, jc).
        # Stage schedule at step t: scores(t), exp(t-1), mask(t-2), AV(t-3)
        # so every instruction's inputs are ready when the engine decodes
        # it (the 4-deep per-engine wait queues otherwise backpressure the
        # sequencers). A(ic+1) projection micro-ops are paced in as filler.
        # ------------------------------------------------------------------
        specs = []
        for ic in range(ic_n):
            for n in range(N):
                for jc in range(4 * ic + 4):
                    specs.append((ic, n, jc))
        nspec = len(specs)
        qt_tiles = {0: qt_next}
        st2_of, at2_of, avs_of, qres_of = {}, {}, {}, {}
        filler = []
        quota = {}

        def emit_filler(k):
            for _ in range(k):
                if filler:
                    filler.pop(0)()

        def stage_scores(idx):
            ic, n, jc = specs[idx]
            st2 = st2p.tile([128, 1024], F32, tag="st2", name="st2")
            st2_of[idx] = st2
            qt_cur = qt_tiles[ic]
            for h in range(H2):
                nc.tensor.matmul(
                    st2[:, h * 512:(h + 1) * 512],
                    kt_sb[h * 64:(h + 1) * 64,
                          n * l + jc * 128:n * l + jc * 128 + 128],
                    qt_cur[h * 64:(h + 1) * 64, n * 512:(n + 1) * 512],
                    start=True, stop=True)

        def stage_exp(idx):
            ic, n, jc = specs[idx]
            rr = jc - 4 * ic
            st2 = st2_of.pop(idx)
            at2 = at2p.tile([128, 1024], BF16, tag="at2", name="at2")
            at2_of[idx] = at2
            if rr <= 0:
                nc.scalar.activation(at2, st2,
                                     mybir.ActivationFunctionType.Exp,
                                     scale=SCALE / (W8_SCALE * W8_SCALE))
            else:
                for h in range(H2):
                    nc.scalar.activation(
                        at2[:, h * 512 + rr * 128:(h + 1) * 512],
                        st2[:, h * 512 + rr * 128:(h + 1) * 512],
                        mybir.ActivationFunctionType.Exp,
                        scale=SCALE / (W8_SCALE * W8_SCALE))

        def stage_mask(idx):
            ic, n, jc = specs[idx]
            rr = jc - 4 * ic
            if rr < 0:
                return
            at2 = at2_of[idx]
            for h in range(H2):
                sl = slice(h * 512 + rr * 128, h * 512 + rr * 128 + 128)
                nc.vector.tensor_mul(at2[:, sl], at2[:, sl], trimask)

        def stage_av(idx):
            ic, n, jc = specs[idx]
            rr = jc - 4 * ic
            at2 = at2_of.pop(idx)
            if jc == 0:
                avs_of[(ic, n)] = [avp.tile([128, 260], F32, tag=f"av{h}",
                                            name=f"av{h}")
                                   for h in range(H2)]
                for h in range(H2):
                    # clear the whole accumulator bank exactly once
                    nc.tensor.matmul(avs_of[(ic, n)][h], zlhs, zrhs,
                                     start=True, stop=True,
                                     skip_group_check=True)
            avs = avs_of[(ic, n)]
            vbase = (n * jblocks + jc) * H2 * 65
            for h in range(H2):
                for S in range(4):
                    if rr > S:
                        continue
                    nc.tensor.matmul(
                        avs[h][:, S * 65:(S + 1) * 65],
                        at2[:, h * 512 + S * 128:h * 512 + S * 128 + 128],
                        v_sb[:, vbase + h * 65:vbase + h * 65 + 65],
                        start=False, stop=(rr == S),
                        skip_group_check=True)
            if jc == 4 * ic + 3:
                enqueue_drain(ic, n)

        # drains and stats run as small deferred pieces, one per iteration,
        # so their engine dependencies are satisfied before dispatch and
        # they never block the in-order PE/DVE queues.
        drain_pending = []

        def enqueue_drain(ic, n):
            avs = avs_of.pop((ic, n))
            qres_t = qres_of.pop((ic, n))
            base512 = (n * 16 + ic * 4) * FC

            def drain_head(h):
                av3 = avs[h].rearrange("p (s x) -> p s x", x=65)
                rec = smallp.tile([128, 4], F32, tag="rec", name="rec")
                nc.vector.reciprocal(rec, av3[:, :, 64])
                for S in range(4):
                    nc.vector.scalar_tensor_tensor(
                        out=res_sb[:, base512 + S * FC + h * 64:
                                   base512 + S * FC + h * 64 + 64],
                        in0=avs[h][:, S * 65:S * 65 + 64],
                        scalar=rec[:, S:S + 1],
                        in1=qres_t[:, S * FC + h * 64:S * FC + h * 64 + 64],
                        op0=mybir.AluOpType.mult,
                        op1=mybir.AluOpType.add)

            def drain_sq():
                res_block = res_sb[:, base512:base512 + 512]
                sqt = sqp.tile([128, 512], BF16, tag="sq", name="sqt")
                nc.vector.tensor_mul(sqt, res_block, res_block)
                stats_bufs[(ic, n)] = (res_block, sqt)

            def drain_stats():
                res_block, sqt = stats_bufs.pop((ic, n))
                first = (n == 0 and ic == 0)
                last = (n == N - 1 and ic == ic_n - 1)
                nc.tensor.matmul(stat[0:1, :], ones_col, res_block,
                                 start=first, stop=last,
                                 skip_group_check=True)
                nc.tensor.matmul(stat[64:65, :], ones_col, sqt,
                                 start=first, stop=last,
                                 skip_group_check=True)

            if DEBUG_NOLAG:
                drain_head(0)
                drain_head(1)
                drain_sq()
                drain_stats()
            else:
                drain_pending.extend(
                    [lambda: drain_head(0), None,
                     lambda: (drain_head(1), drain_sq()), None, drain_stats])

        stats_bufs = {}

        def step_drain(flush=False):
            while drain_pending:
                op = drain_pending.pop(0)
                if op is None:
                    if flush:
                        continue
                    return
                op()

        # chunk 0's remaining projection groups (batches 1-3) are due just
        # before B(0) reaches that batch: group n spread over iters
        # [4(n-1), 4n)
        for n in range(1, N):
            ops = groups0[n]
            no = len(ops)
            for t in range(4):
                quota[4 * (n - 1) + t] = (quota.get(4 * (n - 1) + t, 0)
                                          + ((t + 1) * no) // 4
                                          - (t * no) // 4)
        filler = groups0[1] + groups0[2] + groups0[3]

        for idx in range(nspec + max(EXP_LAG, MASK_LAG, AV_LAG)):
            if idx < nspec:
                ic, n, jc = specs[idx]
                if jc == 0 and n == 0 and ic + 1 < ic_n:
                    # build next chunk's projection fillers, paced over
                    # this chunk's iterations (merged with any deadline
                    # quotas already scheduled for these slots)
                    if ic > 0:
                        emit_filler(len(filler))
                    qt_tiles[ic + 1] = qtp.tile([128, N * 512], BF16,
                                                tag="qt", name="qt")
                    nxq = load_x8(xqp, xq8_nd, 0, ic + 1)
                    nxk8 = load_x8(xk8p, xk8_nd, 0, ic + 1)
                    groups = build_chunk_groups(ic + 1, qt_tiles[ic + 1],
                                                (nxq, nxk8))
                    new_ops = [op for g in groups for op in g]
                    iters = N * (4 * ic + 4)
                    nf = len(new_ops)
                    for t in range(iters):
                        quota[idx + t] = (quota.get(idx + t, 0)
                                          + ((t + 1) * nf) // iters
                                          - (t * nf) // iters)
                    filler.extend(new_ops)
                if jc == 0:
                    qres_t = qinp.tile([128, 4 * FC], BF16, tag="qres",
                                       name="qres")
                    nc.sync.dma_start(
                        qres_t.rearrange("p (s f) -> p s f", s=4),
                        bass.AP(tensor=q_res.tensor,
                                offset=q_res.offset + (n * l + ic * 512) * FC,
                                ap=[[FC, 128], [128 * FC, 4], [1, FC]]))
                    qres_of[(ic, n)] = qres_t
                stage_scores(idx)
            if idx - EXP_LAG >= 0 and idx - EXP_LAG < nspec:
                stage_exp(idx - EXP_LAG)
            if idx - MASK_LAG >= 0 and idx - MASK_LAG < nspec:
                stage_mask(idx - MASK_LAG)
            step_drain()
            if idx - AV_LAG >= 0 and idx - AV_LAG < nspec:
                stage_av(idx - AV_LAG)
            emit_filler(quota.get(idx, 0))
            if idx == nspec - 1:
                emit_filler(len(filler))
        step_drain(flush=True)

        # ------------------------------------------------------------------
        # batch-norm: fold partial sums, compute gamma', beta', apply
        # ------------------------------------------------------------------
        # fold the 4 ls-group partials to [1, FC] on partition 0
        sum_r = bnp.tile([1, 512], F32, tag="sumr", name="sumr")
        nc.vector.tensor_copy(sum_r, stat[0:1, :])
        sq_r = bnp.tile([1, 512], F32, tag="sqr", name="sqr")
        nc.vector.tensor_copy(sq_r, stat[64:65, :])
        sA = bnp.tile([1, FC], F32, tag="sA", name="sA")
        nc.vector.tensor_add(sA, sum_r[:, 0:FC], sum_r[:, FC:2 * FC])
        sB = bnp.tile([1, FC], F32, tag="sB", name="sB")
        nc.vector.tensor_add(sB, sum_r[:, 2 * FC:3 * FC],
                             sum_r[:, 3 * FC:4 * FC])
        sumf = bnp.tile([1, FC], F32, tag="sumf", name="sumf")
        nc.vector.tensor_add(sumf, sA, sB)
        qA = bnp.tile([1, FC], F32, tag="qA", name="qA")
        nc.vector.tensor_add(qA, sq_r[:, 0:FC], sq_r[:, FC:2 * FC])
        qB = bnp.tile([1, FC], F32, tag="qB", name="qB")
        nc.vector.tensor_add(qB, sq_r[:, 2 * FC:3 * FC],
                             sq_r[:, 3 * FC:4 * FC])
        sqf = bnp.tile([1, FC], F32, tag="sqf", name="sqf")
        nc.vector.tensor_add(sqf, qA, qB)

        inv = 1.0 / NL
        mean = bnp.tile([1, FC], F32, tag="mean", name="mean")
        nc.vector.tensor_scalar_mul(mean, sumf, inv)
        musq = bnp.tile([1, FC], F32, tag="musq", name="musq")   # mean^2
        nc.vector.tensor_mul(musq, mean, mean)
        var = bnp.tile([1, FC], F32, tag="var", name="var")
        nc.vector.scalar_tensor_tensor(
            out=var, in0=sqf, scalar=inv, in1=musq,
            op0=mybir.AluOpType.mult, op1=mybir.AluOpType.subtract)
        std = bnp.tile([1, FC], F32, tag="std", name="std")
        nc.scalar.activation(std, var, mybir.ActivationFunctionType.Sqrt,
                             bias=eps_sb[0:1, :])
        rstd = bnp.tile([1, FC], F32, tag="rstd", name="rstd")
        nc.vector.reciprocal(rstd, std)
        gp = bnp.tile([1, FC], F32, tag="gp", name="gp")
        nc.vector.tensor_mul(gp, gamma_sb, rstd)
        mgp = bnp.tile([1, FC], F32, tag="mgp", name="mgp")
        nc.vector.tensor_mul(mgp, mean, gp)
        bp = bnp.tile([1, FC], F32, tag="bp", name="bp")
        nc.vector.tensor_sub(bp, beta_sb, mgp)
        gp16 = bnp.tile([1, FC], BF16, tag="gp16", name="gp16")
        nc.vector.tensor_copy(gp16, gp)
        bp16 = bnp.tile([1, FC], BF16, tag="bp16", name="bp16")
        nc.vector.tensor_copy(bp16, bp)

        gbc = bnp.tile([128, FC], BF16, tag="gbc", name="gbc")
        nc.gpsimd.partition_broadcast(gbc, gp16)
        bbc = bnp.tile([128, FC], BF16, tag="bbc", name="bbc")
        nc.gpsimd.partition_broadcast(bbc, bp16)

        def rep4(t):
            return bass.AP(tensor=t.tensor, offset=t.offset,
                           ap=[[t.ap[0][0], 128], [0, 4], [1, FC]])

        gbc4 = bnp.tile([128, 512], BF16, tag="gbc4", name="gbc4")
        nc.vector.tensor_copy(gbc4, rep4(gbc))
        bbc4 = bnp.tile([128, 512], BF16, tag="bbc4", name="bbc4")
        nc.vector.tensor_copy(bbc4, rep4(bbc))

        for n in range(N):
            for ic in range(ic_n):
                base512 = (n * 16 + ic * 4) * FC
                t1 = outp.tile([128, 512], BF16, tag="t1", name="t1")
                nc.vector.tensor_mul(t1, res_sb[:, base512:base512 + 512],
                                     gbc4)
                t2 = outp.tile([128, 512], BF16, tag="t2", name="t2")
                nc.vector.tensor_add(t2, t1, bbc4)
                nc.sync.dma_start(
                    bass.AP(tensor=out_s.tensor,
                            offset=out_s.offset + (n * l + ic * 512) * FC,
                            ap=[[FC, 128], [128 * FC, 4], [1, FC]]),
                    t2.rearrange("p (s f) -> p s f", s=4))

    nc.compile()
    return nc


def get_runner(nc):
    """Build (once) a cached jitted SPMD executor for the Bass program."""
    if "runner" in _cached:
        return _cached["runner"]

    import jax
    from jax.experimental.shard_map import shard_map
    from jax.sharding import Mesh, PartitionSpec
    from concourse import bass2jax

    bass2jax.install_neuronx_cc_hook()

    partition_name = (nc.partition_id_tensor.name
                      if nc.partition_id_tensor else None)
    in_names, out_names, out_avals, zero_outs = [], [], [], []
    for alloc in nc.m.functions[0].allocations:
        if not isinstance(alloc, mybir.MemoryLocationSet):
            continue
        name = alloc.memorylocations[0].name
        if alloc.kind == "ExternalInput":
            if name != partition_name:
                in_names.append(name)
        elif alloc.kind == "ExternalOutput":
            shape = tuple(alloc.tensor_shape)
            dtype = mybir.dt.np(alloc.dtype)
            out_names.append(name)
            out_avals.append(jax.core.ShapedArray(shape, dtype))
            zero_outs.append(np.zeros(shape, dtype))
    n_params = len(in_names)
    n_outs = len(out_avals)
    all_names = in_names + out_names
    if partition_name is not None:
        all_names = all_names + [partition_name]

    def _body(*args):
        operands = list(args)
        if partition_name is not None:
            operands.append(bass2jax.partition_id_tensor())
        outs = bass2jax._bass_exec_p.bind(
            *operands,
            out_avals=tuple(out_avals),
            in_names=tuple(all_names),
            out_names=tuple(out_names),
            lowering_input_output_aliases=(),
            sim_require_finite=True,
            sim_require_nnan=True,
            nc=nc,
        )
        return tuple(outs)

    devices = jax.devices()[:NCORES]
    mesh = Mesh(np.asarray(devices), ("core",))
    in_specs = (PartitionSpec("core"),) * (n_params + n_outs)
    out_specs = (PartitionSpec("core"),) * n_outs
    donate = tuple(range(n_params, n_params + n_outs))
    sharded = jax.jit(
        shard_map(_body, mesh=mesh, in_specs=in_specs, out_specs=out_specs,
                  check_rep=False),
        donate_argnums=donate, keep_unused=True)

    def run_np(in_maps):
        concat_in = [
            np.concatenate([np.asarray(in_maps[c][nm]) for c in range(NCORES)],
                           axis=0)
            for nm in in_names]
        concat_zeros = [np.zeros((NCORES * z.shape[0], *z.shape[1:]), z.dtype)
                        for z in zero_outs]
        out_arrs = sharded(*concat_in, *concat_zeros)
        return [
            {nm: np.asarray(out_arrs[i]).reshape(
                NCORES, *out_avals[i].shape)[c]
             for i, nm in enumerate(out_names)}
            for c in range(NCORES)]

    _cached["runner"] = (run_np, sharded, in_names, out_names, out_avals,
                         zero_outs, mesh)
    return _cached["runner"]


def make_in_maps(inputs, l):
    query = np.asarray(inputs["query"], dtype=np.float32)
    key = np.asarray(inputs["key"], dtype=np.float32)
    Wq = np.asarray(inputs["Wq"], dtype=np.float32)
    Wk = np.asarray(inputs["Wk"], dtype=np.float32)
    Wv = np.asarray(inputs["Wv"], dtype=np.float32)
    gamma = np.asarray(inputs["gamma"], dtype=np.float32)
    beta = np.asarray(inputs["beta"], dtype=np.float32)

    n = query.shape[0]
    qf = query.reshape(n * l, D)
    kf = key.reshape(n * l, D)
    xq8 = np.ascontiguousarray(qf.T.astype(FP8_NP))
    xk8 = np.ascontiguousarray(kf.T.astype(FP8_NP))

    in_maps = []
    for c in range(NCORES):
        sl = slice(c * FC, (c + 1) * FC)
        in_maps.append({
            "xq8_nd": xq8,
            "xk8_nd": xk8,
            "wq8": np.ascontiguousarray(
                (Wq[sl].T * W8_SCALE).astype(FP8_NP)),
            "wk8": np.ascontiguousarray(
                (Wk[sl].T * W8_SCALE).astype(FP8_NP)),
            "wvt": np.ascontiguousarray(Wv[sl].T.astype(FP8_NP)),
            "q_res": np.ascontiguousarray(qf[:, sl].astype(BF16_NP)),
            "gamma": np.ascontiguousarray(gamma[sl].reshape(1, FC)),
            "beta": np.ascontiguousarray(beta[sl].reshape(1, FC)),
        })
    return in_maps


def kernel(**inputs):
    l = np.asarray(inputs["query"]).shape[1]
    if "nc" not in _cached or _cached.get("l") != l:
        _cached["nc"] = build_program(l)
        _cached["l"] = l
    nc = _cached["nc"]

    in_maps = make_in_maps(inputs, l)
    run_np = get_runner(nc)[0]
    results = run_np(in_maps)

    n = np.asarray(inputs["query"]).shape[0]
    out = np.zeros((n, l, D), dtype=np.float32)
    for c in range(NCORES):
        sl = slice(c * FC, (c + 1) * FC)
        out[:, :, sl] = results[c]["out_s"].reshape(n, l, FC).astype(
            np.float32)
    return out


# revision 59
# speedup vs baseline: 1.2841x; 1.0287x over previous
"""Trainium2 Bass kernel for MultiHeadAttention + residual + BatchNorm.

Model (reference):
  q = query @ Wq.T ; k = key @ Wk.T ; v = key @ Wv.T    (per-head split)
  score = q k^T / sqrt(D), causal mask, softmax over keys
  res   = (attn @ v) + query
  out   = batchnorm(res over all (N*L) rows, per feature) * gamma + beta

Sharding over 8 cores: FEATURE sharding. Core c owns heads {2c, 2c+1}
(features [128c, 128c+128)) for ALL batches. BatchNorm statistics are
then core-local (sums over all N*L rows of the core's own features), so
no collective is needed at all.

All matmul operands are bf16 (PSUM accumulation stays fp32); the
residual add, batch-norm statistics and outputs are fp32.

Per 512-row query chunk ic and batch n the attention inner loop walks
key blocks jc (128 keys each, causal): PE computes both heads' scores
into one PSUM tile [128 j, 1024 (h,i)], ScalarE applies a single merged
exp -> bf16, DVE masks the diagonal block with a triangular multiply,
and PE accumulates the *flipped* attention-V product out[i, p] with a
ones-column appended to V so softmax denominators fall out of the same
matmuls. Projections for chunk ic+1 are emitted as small PE micro-ops
paced between attention iterations so the scalar engine never starves.
"""

import math
import sys

sys.path.insert(0, "/opt/trn_rl_repo")

import numpy as np
import ml_dtypes

import concourse.bass as bass
import concourse.mybir as mybir
from concourse import bacc
import concourse.tile as tile

F32 = mybir.dt.float32
F32R = mybir.dt.float32r
BF16 = mybir.dt.bfloat16
FP8 = mybir.dt.float8e4
BF16_NP = ml_dtypes.bfloat16
FP8_NP = mybir.dt.np(FP8)
# q/k weights are scaled by 16 on the host so fp8e4 stays out of the
# subnormal range; q.k scores come out 256x large, compensated in the
# exp's scale argument
W8_SCALE = 16.0

N = 4
L = 2048
D = 1024
H = 16
P = 64
NCORES = 8
FC = D // NCORES       # features per core = 128
H2 = 2                 # heads per core
EPS = 1e-5
SCALE = 1.0 / math.sqrt(D)
NL = N * L             # 8192 rows in the global batch norm

# software-pipeline stage lags (consumers trail producers so engine wait
# queues never backpressure the sequencers)
EXP_LAG = 1
MASK_LAG = 2
AV_LAG = 3
DEBUG_NOLAG = False

_cached = {}


def r(ap):
    return ap.bitcast(F32R)


def build_program(l=L):
    """Build the SPMD Bass program (identical on all 8 cores)."""
    nc = bacc.Bacc("TRN2", target_bir_lowering=False, debug=False,
                   num_devices=NCORES)

    ic_n = l // 512        # 512-row query chunks
    nlc = N * l            # rows per core (all batches)

    xq8_nd = nc.dram_tensor("xq8_nd", [D, nlc], FP8,
                            kind="ExternalInput").ap()
    xk8_nd = nc.dram_tensor("xk8_nd", [D, nlc], FP8,
                            kind="ExternalInput").ap()
    wq8 = nc.dram_tensor("wq8", [D, FC], FP8, kind="ExternalInput").ap()
    wk8 = nc.dram_tensor("wk8", [D, FC], FP8, kind="ExternalInput").ap()
    wvt = nc.dram_tensor("wvt", [D, FC], FP8, kind="ExternalInput").ap()
    q_res = nc.dram_tensor("q_res", [nlc, FC], BF16, kind="ExternalInput").ap()
    gamma = nc.dram_tensor("gamma", [1, FC], F32, kind="ExternalInput").ap()
    beta = nc.dram_tensor("beta", [1, FC], F32, kind="ExternalInput").ap()
    out_s = nc.dram_tensor("out_s", [nlc, FC], BF16,
                           kind="ExternalOutput").ap()

    jblocks = l // 128     # 16 key blocks per batch
    nls = nlc // 128       # 64 ls blocks of res

    from contextlib import ExitStack
    with ExitStack() as stack:
        tc = stack.enter_context(tile.TileContext(nc))
        pool = {}
        for nm, bufs, space in (
                ("consts", 1, None), ("persist", 1, None), ("wt", 1, None),
                ("qtp", 2, None), ("xq", 2, None), ("xk", 2, None),
                ("xk8", 2, None),
                ("at2", 3, None), ("qin", 2, None), ("sq", 2, None),
                ("outp", 3, None), ("bnp", 1, None), ("small", 6, None),
                ("st2", 2, "PSUM"), ("av", 1, "PSUM"), ("pj", 1, "PSUM"),
                ("stat", 1, "PSUM")):
            kw = {"name": nm, "bufs": bufs}
            if space:
                kw["space"] = space
            pool[nm] = stack.enter_context(tc.tile_pool(**kw))
        consts, persist, wtp = pool["consts"], pool["persist"], pool["wt"]
        qtp, xqp, xkp = pool["qtp"], pool["xq"], pool["xk"]
        xk8p = pool["xk8"]
        at2p, qinp, sqp = pool["at2"], pool["qin"], pool["sq"]
        outp, bnp, smallp = pool["outp"], pool["bnp"], pool["small"]
        st2p, avp, pjp, statp = (pool["st2"], pool["av"], pool["pj"],
                                 pool["stat"])

        # ---------------- first activation chunk DMAs (critical path) ----
        def x_src(dram, n, ic):
            return bass.AP(
                tensor=dram.tensor,
                offset=dram.offset + n * l + ic * 512,
                ap=[[nlc, 128], [128 * nlc, 8], [1, 512]])

        def load_x(pool, dram, n, ic):
            t = pool.tile([128, 8 * 512], BF16, tag="x", name="xt")
            nc.sync.dma_start(
                t.rearrange("p (dc x) -> p dc x", dc=8), x_src(dram, n, ic))
            return t

        def x8_src(dram, n, ic):
            # d = s*256 + t*128 + p (fp8 DoubleRow pair layout)
            return bass.AP(
                tensor=dram.tensor,
                offset=dram.offset + n * l + ic * 512,
                ap=[[nlc, 128], [256 * nlc, 4], [128 * nlc, 2], [1, 512]])

        def load_x8(pool, dram, n, ic):
            t = pool.tile([128, 4 * 2 * 512], FP8, tag="x8", name="x8t")
            nc.sync.dma_start(
                t.rearrange("p (s t x) -> p s t x", s=4, t=2),
                x8_src(dram, n, ic))
            return t

        # first q-projection chain needs wq then xq: emit those two DMAs
        # first so PE can start as early as possible
        wts = {}

        def load_w8(wname, wdram):
            t = wtp.tile([128, 4 * 2 * FC], FP8, tag=wname, name=wname)
            nc.sync.dma_start(
                t.rearrange("p (s t f) -> p s t f", s=4, t=2),
                bass.AP(tensor=wdram.tensor, offset=wdram.offset,
                        ap=[[FC, 128], [256 * FC, 4], [128 * FC, 2],
                            [1, FC]]))
            wts[wname] = t

        load_w8("wq", wq8)
        xq_t = load_x8(xqp, xq8_nd, 0, 0)
        load_w8("wk", wk8)
        xk8_t = load_x8(xk8p, xk8_nd, 0, 0)
        load_w8("wv", wvt)

        # ---------------- constants -------------------------------------
        ones_col = consts.tile([128, 1], BF16)
        nc.vector.memset(ones_col, 1.0)
        eps_sb = consts.tile([128, 1], F32)
        nc.vector.memset(eps_sb, EPS)
        gamma_sb = consts.tile([1, FC], F32)
        nc.sync.dma_start(gamma_sb, gamma)
        beta_sb = consts.tile([1, FC], F32)
        nc.sync.dma_start(beta_sb, beta)
        # lower-triangular (j <= i) mask in [j-part, i-free] layout
        tm_f = consts.tile([128, 128], F32)
        nc.vector.memset(tm_f, 1.0)
        nc.gpsimd.affine_select(
            out=tm_f, in_=tm_f,
            compare_op=mybir.AluOpType.is_ge, fill=0.0, base=0,
            pattern=[[1, 128]], channel_multiplier=-1)
        trimask = consts.tile([128, 128], BF16)
        nc.vector.tensor_copy(trimask, tm_f)
        # zero operands for the PSUM-bank-clearing matmuls (PSUM
        # start_tensor_calc marks a whole 2KB zero-region, so banks shared
        # by several accumulation groups must be cleared by one explicit
        # full-tile start matmul instead of per-group start flags)
        zlhs = consts.tile([128, 128], BF16)
        nc.vector.memset(zlhs, 0.0)
        zrhs = consts.tile([128, 260], BF16)
        nc.vector.memset(zrhs, 0.0)

        # ---------------- persistent SBUF -------------------------------
        # kt_sb: [feature(h*64+p), (n, j)] bf16
        kt_sb = persist.tile([128, N * l], BF16, tag="kt")
        # v_sb: [j-in-block, (n, jc, h, 65)] bf16; col 64 of each 65-group
        # is the baked ones column (softmax denominator trick)
        v_sb = persist.tile([128, N * jblocks * H2 * 65], BF16, tag="v")
        v3 = v_sb.rearrange("p (g x) -> p g x", x=65)
        nc.gpsimd.memset(v3[:, :, 64:65], 1.0)
        # res_sb: [l-in-block, (ls, f)] bf16, ls = n*16 + ic*4 + S
        res_sb = persist.tile([128, nls * FC], BF16, tag="res")

        # partition 0 row: sums; partition 64 row: sums of squares
        # (matmul outputs must start at partition 0, 32, or 64)
        stat = statp.tile([65, 512], F32, tag="stat")

        # ------------------------------------------------------------------
        # projection task machinery (filler micro-ops paced into B loops)
        # ------------------------------------------------------------------
        # rotating PSUM allocators: the interleaved fillers use the single
        # pj bank; the up-front A(0) block also rotates through the two
        # (then idle) score banks so chains overlap their copy-out
        def alloc_pj():
            return pjp.tile([128, 512], F32, tag="pj", name="pj")

        def alloc_st2_slot():
            return st2p.tile([128, 1024], F32, tag="st2", name="st2")[:, 0:512]

        rot = {"i": 0}

        def alloc_rotating():
            rot["i"] += 1
            return alloc_pj() if rot["i"] % 3 == 0 else alloc_st2_slot()

        chain_alloc = {"fn": alloc_pj}

        def qk_chain(side, n, ic, qt_tile, get_xt):
            """q/k projection via fp8 DoubleRow: 4 contraction-256 steps."""
            w_use = wts["wq"] if side == "q" else wts["wk"]
            w4 = w_use.rearrange("p (s t f) -> p s t f", s=4, t=2)
            pj = {}
            alloc_fn = chain_alloc["fn"]

            def alloc():
                pj["t"] = alloc_fn()

            def mm(s0):
                x4 = get_xt().rearrange("p (s t x) -> p s t x", s=4, t=2)
                for s in (s0, s0 + 1):
                    nc.tensor.matmul(
                        pj["t"], w4[:, s], x4[:, s],
                        start=(s == 0), stop=(s == 3),
                        perf_mode=mybir.MatmulPerfMode.DoubleRow)

            def copy():
                if side == "q":
                    nc.vector.tensor_copy(qt_tile[:, n * 512:(n + 1) * 512],
                                          pj["t"])
                else:
                    nc.vector.tensor_copy(
                        kt_sb[:, n * l + ic * 512:n * l + ic * 512 + 512],
                        pj["t"])

            ops = [alloc]
            for s0 in range(0, 4, 2):
                ops.append(lambda s=s0: mm(s))
            ops.append(copy)
            return ops

        def wts_slice(w, dc):
            return w[:, dc * FC:(dc + 1) * FC]

        def v_chain(n, jsub, ic, get_xt):
            pj = {}
            alloc_fn = chain_alloc["fn"]
            wv4 = wts["wv"].rearrange("p (s t f) -> p s t f", s=4, t=2)

            def alloc():
                pj["t"] = alloc_fn()

            def mm(s0):
                x4 = get_xt().rearrange("p (s t x) -> p s t x", s=4, t=2)
                for s in (s0, s0 + 1):
                    nc.tensor.matmul(
                        pj["t"][:, 0:128],
                        x4[:, s, :, jsub * 128:jsub * 128 + 128],
                        wv4[:, s],
                        start=(s == 0), stop=(s == 3),
                        perf_mode=mybir.MatmulPerfMode.DoubleRow)

            def copy():
                jc = ic * 4 + jsub
                base = (n * jblocks + jc) * H2 * 65
                dst = v_sb[:, base:base + 130].rearrange(
                    "p (h x) -> p h x", h=2)[:, :, 0:64]
                src = pj["t"][:, 0:128].rearrange("p (h x) -> p h x", h=2)
                nc.vector.tensor_copy(dst, src)

            ops = [alloc]
            for s0 in range(0, 4, 2):
                ops.append(lambda s=s0: mm(s))
            ops.append(copy)
            return ops

        def build_chunk_groups(ic, qt_tile, first_x, rotate_first=False):
            """Return per-batch lists of micro-op closures for A(ic)."""
            groups = []
            xq_cur = {0: first_x[0]}
            xk8_cur = {0: first_x[1]}
            for n in range(N):
                chain_alloc["fn"] = (alloc_rotating if rotate_first and n == 0
                                     else alloc_pj)
                ops = []
                get_xq = lambda nn=n: xq_cur[nn]
                get_xk8 = lambda nn=n: xk8_cur[nn]
                # prefetch next batch's activations
                if n + 1 < N:
                    def pre(nn=n + 1):
                        xq_cur[nn] = load_x8(xqp, xq8_nd, nn, ic)
                        xk8_cur[nn] = load_x8(xk8p, xk8_nd, nn, ic)
                    ops.append(pre)
                ops += qk_chain("q", n, ic, qt_tile, get_xq)
                ops += qk_chain("k", n, ic, None, get_xk8)
                for jsub in range(4):
                    ops += v_chain(n, jsub, ic, get_xk8)
                groups.append(ops)
            return groups

        # ------------------------------------------------------------------
        # A(0): only batch 0's projections run up front; batches 1-3 are
        # deadline-paced into B(0)'s iterations (batch n is needed at
        # B(0) iteration 4n) so the scalar engine starts exp'ing early.
        # ------------------------------------------------------------------
        qt_next = qtp.tile([128, N * 512], BF16, tag="qt", name="qt")
        groups0 = build_chunk_groups(0, qt_next, (xq_t, xk8_t),
                                     rotate_first=True)
        for op in groups0[0]:
            op()

        # ------------------------------------------------------------------
        # main loop: one software-pipelined stream over (ic, n, jc).
        # Stage schedule at step t: scores(t), exp(t-1), mask(t-2), AV(t-3)
        # so every instruction's inputs are ready when the engine decodes
        # it (the 4-deep per-engine wait queues otherwise backpressure the
        # sequencers). A(ic+1) projection micro-ops are paced in as filler.
        # ------------------------------------------------------------------
        specs = []
        for ic in range(ic_n):
            for n in range(N):
                for jc in range(4 * ic + 4):
                    specs.append((ic, n, jc))
        nspec = len(specs)
        qt_tiles = {0: qt_next}
        st2_of, at2_of, avs_of, qres_of = {}, {}, {}, {}
        filler = []
        quota = {}

        def emit_filler(k):
            for _ in range(k):
                if filler:
                    filler.pop(0)()

        def stage_scores(idx):
            ic, n, jc = specs[idx]
            st2 = st2p.tile([128, 1024], F32, tag="st2", name="st2")
            st2_of[idx] = st2
            qt_cur = qt_tiles[ic]
            for h in range(H2):
                nc.tensor.matmul(
                    st2[:, h * 512:(h + 1) * 512],
                    kt_sb[h * 64:(h + 1) * 64,
                          n * l + jc * 128:n * l + jc * 128 + 128],
                    qt_cur[h * 64:(h + 1) * 64, n * 512:(n + 1) * 512],
                    start=True, stop=True)

        def stage_exp(idx):
            ic, n, jc = specs[idx]
            rr = jc - 4 * ic
            st2 = st2_of.pop(idx)
            at2 = at2p.tile([128, 1024], BF16, tag="at2", name="at2")
            at2_of[idx] = at2
            if rr <= 0:
                nc.scalar.activation(at2, st2,
                                     mybir.ActivationFunctionType.Exp,
                                     scale=SCALE / (W8_SCALE * W8_SCALE))
            else:
                # one strided call covering both heads' causal windows
                a3 = at2.rearrange("p (h x) -> p h x", h=2)[:, :, rr * 128:]
                s3 = st2.rearrange("p (h x) -> p h x", h=2)[:, :, rr * 128:]
                nc.scalar.activation(
                    a3, s3, mybir.ActivationFunctionType.Exp,
                    scale=SCALE / (W8_SCALE * W8_SCALE))

        def stage_mask(idx):
            ic, n, jc = specs[idx]
            rr = jc - 4 * ic
            if rr < 0:
                return
            at2 = at2_of[idx]
            for h in range(H2):
                sl = slice(h * 512 + rr * 128, h * 512 + rr * 128 + 128)
                nc.vector.tensor_mul(at2[:, sl], at2[:, sl], trimask)

        def stage_av(idx):
            ic, n, jc = specs[idx]
            rr = jc - 4 * ic
            at2 = at2_of.pop(idx)
            if jc == 0:
                avs_of[(ic, n)] = [avp.tile([128, 260], F32, tag=f"av{h}",
                                            name=f"av{h}")
                                   for h in range(H2)]
                for h in range(H2):
                    # clear the whole accumulator bank exactly once
                    nc.tensor.matmul(avs_of[(ic, n)][h], zlhs, zrhs,
                                     start=True, stop=True,
                                     skip_group_check=True)
            avs = avs_of[(ic, n)]
            vbase = (n * jblocks + jc) * H2 * 65
            for h in range(H2):
                for S in range(4):
                    if rr > S:
                        continue
                    nc.tensor.matmul(
                        avs[h][:, S * 65:(S + 1) * 65],
                        at2[:, h * 512 + S * 128:h * 512 + S * 128 + 128],
                        v_sb[:, vbase + h * 65:vbase + h * 65 + 65],
                        start=False, stop=(rr == S),
                        skip_group_check=True)
            if jc == 4 * ic + 3:
                enqueue_drain(ic, n)

        # drains and stats run as small deferred pieces, one per iteration,
        # so their engine dependencies are satisfied before dispatch and
        # they never block the in-order PE/DVE queues.
        drain_pending = []

        def enqueue_drain(ic, n):
            avs = avs_of.pop((ic, n))
            qres_t = qres_of.pop((ic, n))
            base512 = (n * 16 + ic * 4) * FC

            def drain_head(h):
                av3 = avs[h].rearrange("p (s x) -> p s x", x=65)
                rec = smallp.tile([128, 4], F32, tag="rec", name="rec")
                nc.vector.reciprocal(rec, av3[:, :, 64])
                for S in range(4):
                    nc.vector.scalar_tensor_tensor(
                        out=res_sb[:, base512 + S * FC + h * 64:
                                   base512 + S * FC + h * 64 + 64],
                        in0=avs[h][:, S * 65:S * 65 + 64],
                        scalar=rec[:, S:S + 1],
                        in1=qres_t[:, S * FC + h * 64:S * FC + h * 64 + 64],
                        op0=mybir.AluOpType.mult,
                        op1=mybir.AluOpType.add)

            def drain_sq():
                res_block = res_sb[:, base512:base512 + 512]
                sqt = sqp.tile([128, 512], BF16, tag="sq", name="sqt")
                nc.vector.tensor_mul(sqt, res_block, res_block)
                stats_bufs[(ic, n)] = (res_block, sqt)

            def drain_stats():
                res_block, sqt = stats_bufs.pop((ic, n))
                first = (n == 0 and ic == 0)
                last = (n == N - 1 and ic == ic_n - 1)
                nc.tensor.matmul(stat[0:1, :], ones_col, res_block,
                                 start=first, stop=last,
                                 skip_group_check=True)
                nc.tensor.matmul(stat[64:65, :], ones_col, sqt,
                                 start=first, stop=last,
                                 skip_group_check=True)

            if DEBUG_NOLAG:
                drain_head(0)
                drain_head(1)
                drain_sq()
                drain_stats()
            else:
                drain_pending.extend(
                    [lambda: drain_head(0), None,
                     lambda: (drain_head(1), drain_sq()), None, drain_stats])

        stats_bufs = {}

        def step_drain(flush=False):
            while drain_pending:
                op = drain_pending.pop(0)
                if op is None:
                    if flush:
                        continue
                    return
                op()

        # chunk 0's remaining projection groups (batches 1-3) are due just
        # before B(0) reaches that batch: group n spread over iters
        # [4(n-1), 4n)
        for n in range(1, N):
            ops = groups0[n]
            no = len(ops)
            for t in range(4):
                quota[4 * (n - 1) + t] = (quota.get(4 * (n - 1) + t, 0)
                                          + ((t + 1) * no) // 4
                                          - (t * no) // 4)
        filler = groups0[1] + groups0[2] + groups0[3]

        for idx in range(nspec + max(EXP_LAG, MASK_LAG, AV_LAG)):
            if idx < nspec:
                ic, n, jc = specs[idx]
                if jc == 0 and n == 0 and ic + 1 < ic_n:
                    # build next chunk's projection fillers, paced over
                    # this chunk's iterations (merged with any deadline
                    # quotas already scheduled for these slots)
                    if ic > 0:
                        emit_filler(len(filler))
                    qt_tiles[ic + 1] = qtp.tile([128, N * 512], BF16,
                                                tag="qt", name="qt")
                    nxq = load_x8(xqp, xq8_nd, 0, ic + 1)
                    nxk8 = load_x8(xk8p, xk8_nd, 0, ic + 1)
                    groups = build_chunk_groups(ic + 1, qt_tiles[ic + 1],
                                                (nxq, nxk8))
                    new_ops = [op for g in groups for op in g]
                    iters = N * (4 * ic + 4)
                    nf = len(new_ops)
                    for t in range(iters):
                        quota[idx + t] = (quota.get(idx + t, 0)
                                          + ((t + 1) * nf) // iters
                                          - (t * nf) // iters)
                    filler.extend(new_ops)
                if jc == 0:
                    qres_t = qinp.tile([128, 4 * FC], BF16, tag="qres",
                                       name="qres")
                    nc.sync.dma_start(
                        qres_t.rearrange("p (s f) -> p s f", s=4),
                        bass.AP(tensor=q_res.tensor,
                                offset=q_res.offset + (n * l + ic * 512) * FC,
                                ap=[[FC, 128], [128 * FC, 4], [1, FC]]))
                    qres_of[(ic, n)] = qres_t
                stage_scores(idx)
            if idx - EXP_LAG >= 0 and idx - EXP_LAG < nspec:
                stage_exp(idx - EXP_LAG)
            if idx - MASK_LAG >= 0 and idx - MASK_LAG < nspec:
                stage_mask(idx - MASK_LAG)
            step_drain()
            if idx - AV_LAG >= 0 and idx - AV_LAG < nspec:
                stage_av(idx - AV_LAG)
            emit_filler(quota.get(idx, 0))
            if idx == nspec - 1:
                emit_filler(len(filler))
        step_drain(flush=True)

        # ------------------------------------------------------------------
        # batch-norm: fold partial sums, compute gamma', beta', apply
        # ------------------------------------------------------------------
        # fold the 4 ls-group partials to [1, FC] on partition 0
        sum_r = bnp.tile([1, 512], F32, tag="sumr", name="sumr")
        nc.vector.tensor_copy(sum_r, stat[0:1, :])
        sq_r = bnp.tile([1, 512], F32, tag="sqr", name="sqr")
        nc.vector.tensor_copy(sq_r, stat[64:65, :])
        sA = bnp.tile([1, FC], F32, tag="sA", name="sA")
        nc.vector.tensor_add(sA, sum_r[:, 0:FC], sum_r[:, FC:2 * FC])
        sB = bnp.tile([1, FC], F32, tag="sB", name="sB")
        nc.vector.tensor_add(sB, sum_r[:, 2 * FC:3 * FC],
                             sum_r[:, 3 * FC:4 * FC])
        sumf = bnp.tile([1, FC], F32, tag="sumf", name="sumf")
        nc.vector.tensor_add(sumf, sA, sB)
        qA = bnp.tile([1, FC], F32, tag="qA", name="qA")
        nc.vector.tensor_add(qA, sq_r[:, 0:FC], sq_r[:, FC:2 * FC])
        qB = bnp.tile([1, FC], F32, tag="qB", name="qB")
        nc.vector.tensor_add(qB, sq_r[:, 2 * FC:3 * FC],
                             sq_r[:, 3 * FC:4 * FC])
        sqf = bnp.tile([1, FC], F32, tag="sqf", name="sqf")
        nc.vector.tensor_add(sqf, qA, qB)

        inv = 1.0 / NL
        mean = bnp.tile([1, FC], F32, tag="mean", name="mean")
        nc.vector.tensor_scalar_mul(mean, sumf, inv)
        musq = bnp.tile([1, FC], F32, tag="musq", name="musq")   # mean^2
        nc.vector.tensor_mul(musq, mean, mean)
        var = bnp.tile([1, FC], F32, tag="var", name="var")
        nc.vector.scalar_tensor_tensor(
            out=var, in0=sqf, scalar=inv, in1=musq,
            op0=mybir.AluOpType.mult, op1=mybir.AluOpType.subtract)
        std = bnp.tile([1, FC], F32, tag="std", name="std")
        nc.scalar.activation(std, var, mybir.ActivationFunctionType.Sqrt,
                             bias=eps_sb[0:1, :])
        rstd = bnp.tile([1, FC], F32, tag="rstd", name="rstd")
        nc.vector.reciprocal(rstd, std)
        gp = bnp.tile([1, FC], F32, tag="gp", name="gp")
        nc.vector.tensor_mul(gp, gamma_sb, rstd)
        mgp = bnp.tile([1, FC], F32, tag="mgp", name="mgp")
        nc.vector.tensor_mul(mgp, mean, gp)
        bp = bnp.tile([1, FC], F32, tag="bp", name="bp")
        nc.vector.tensor_sub(bp, beta_sb, mgp)
        gp16 = bnp.tile([1, FC], BF16, tag="gp16", name="gp16")
        nc.vector.tensor_copy(gp16, gp)
        bp16 = bnp.tile([1, FC], BF16, tag="bp16", name="bp16")
        nc.vector.tensor_copy(bp16, bp)

        gbc = bnp.tile([128, FC], BF16, tag="gbc", name="gbc")
        nc.gpsimd.partition_broadcast(gbc, gp16)
        bbc = bnp.tile([128, FC], BF16, tag="bbc", name="bbc")
        nc.gpsimd.partition_broadcast(bbc, bp16)

        def rep4(t):
            return bass.AP(tensor=t.tensor, offset=t.offset,
                           ap=[[t.ap[0][0], 128], [0, 4], [1, FC]])

        gbc4 = bnp.tile([128, 512], BF16, tag="gbc4", name="gbc4")
        nc.vector.tensor_copy(gbc4, rep4(gbc))
        bbc4 = bnp.tile([128, 512], BF16, tag="bbc4", name="bbc4")
        nc.vector.tensor_copy(bbc4, rep4(bbc))

        for n in range(N):
            for ic in range(ic_n):
                base512 = (n * 16 + ic * 4) * FC
                t1 = outp.tile([128, 512], BF16, tag="t1", name="t1")
                nc.vector.tensor_mul(t1, res_sb[:, base512:base512 + 512],
                                     gbc4)
                t2 = outp.tile([128, 512], BF16, tag="t2", name="t2")
                nc.vector.tensor_add(t2, t1, bbc4)
                nc.sync.dma_start(
                    bass.AP(tensor=out_s.tensor,
                            offset=out_s.offset + (n * l + ic * 512) * FC,
                            ap=[[FC, 128], [128 * FC, 4], [1, FC]]),
                    t2.rearrange("p (s f) -> p s f", s=4))

    nc.compile()
    return nc


def get_runner(nc):
    """Build (once) a cached jitted SPMD executor for the Bass program."""
    if "runner" in _cached:
        return _cached["runner"]

    import jax
    from jax.experimental.shard_map import shard_map
    from jax.sharding import Mesh, PartitionSpec
    from concourse import bass2jax

    bass2jax.install_neuronx_cc_hook()

    partition_name = (nc.partition_id_tensor.name
                      if nc.partition_id_tensor else None)
    in_names, out_names, out_avals, zero_outs = [], [], [], []
    for alloc in nc.m.functions[0].allocations:
        if not isinstance(alloc, mybir.MemoryLocationSet):
            continue
        name = alloc.memorylocations[0].name
        if alloc.kind == "ExternalInput":
            if name != partition_name:
                in_names.append(name)
        elif alloc.kind == "ExternalOutput":
            shape = tuple(alloc.tensor_shape)
            dtype = mybir.dt.np(alloc.dtype)
            out_names.append(name)
            out_avals.append(jax.core.ShapedArray(shape, dtype))
            zero_outs.append(np.zeros(shape, dtype))
    n_params = len(in_names)
    n_outs = len(out_avals)
    all_names = in_names + out_names
    if partition_name is not None:
        all_names = all_names + [partition_name]

    def _body(*args):
        operands = list(args)
        if partition_name is not None:
            operands.append(bass2jax.partition_id_tensor())
        outs = bass2jax._bass_exec_p.bind(
            *operands,
            out_avals=tuple(out_avals),
            in_names=tuple(all_names),
            out_names=tuple(out_names),
            lowering_input_output_aliases=(),
            sim_require_finite=True,
            sim_require_nnan=True,
            nc=nc,
        )
        return tuple(outs)

    devices = jax.devices()[:NCORES]
    mesh = Mesh(np.asarray(devices), ("core",))
    in_specs = (PartitionSpec("core"),) * (n_params + n_outs)
    out_specs = (PartitionSpec("core"),) * n_outs
    donate = tuple(range(n_params, n_params + n_outs))
    sharded = jax.jit(
        shard_map(_body, mesh=mesh, in_specs=in_specs, out_specs=out_specs,
                  check_rep=False),
        donate_argnums=donate, keep_unused=True)

    def run_np(in_maps):
        concat_in = [
            np.concatenate([np.asarray(in_maps[c][nm]) for c in range(NCORES)],
                           axis=0)
            for nm in in_names]
        concat_zeros = [np.zeros((NCORES * z.shape[0], *z.shape[1:]), z.dtype)
                        for z in zero_outs]
        out_arrs = sharded(*concat_in, *concat_zeros)
        return [
            {nm: np.asarray(out_arrs[i]).reshape(
                NCORES, *out_avals[i].shape)[c]
             for i, nm in enumerate(out_names)}
            for c in range(NCORES)]

    _cached["runner"] = (run_np, sharded, in_names, out_names, out_avals,
                         zero_outs, mesh)
    return _cached["runner"]


def make_in_maps(inputs, l):
    query = np.asarray(inputs["query"], dtype=np.float32)
    key = np.asarray(inputs["key"], dtype=np.float32)
    Wq = np.asarray(inputs["Wq"], dtype=np.float32)
    Wk = np.asarray(inputs["Wk"], dtype=np.float32)
    Wv = np.asarray(inputs["Wv"], dtype=np.float32)
    gamma = np.asarray(inputs["gamma"], dtype=np.float32)
    beta = np.asarray(inputs["beta"], dtype=np.float32)

    n = query.shape[0]
    qf = query.reshape(n * l, D)
    kf = key.reshape(n * l, D)
    xq8 = np.ascontiguousarray(qf.T.astype(FP8_NP))
    xk8 = np.ascontiguousarray(kf.T.astype(FP8_NP))

    in_maps = []
    for c in range(NCORES):
        sl = slice(c * FC, (c + 1) * FC)
        in_maps.append({
            "xq8_nd": xq8,
            "xk8_nd": xk8,
            "wq8": np.ascontiguousarray(
                (Wq[sl].T * W8_SCALE).astype(FP8_NP)),
            "wk8": np.ascontiguousarray(
                (Wk[sl].T * W8_SCALE).astype(FP8_NP)),
            "wvt": np.ascontiguousarray(Wv[sl].T.astype(FP8_NP)),
            "q_res": np.ascontiguousarray(qf[:, sl].astype(BF16_NP)),
            "gamma": np.ascontiguousarray(gamma[sl].reshape(1, FC)),
            "beta": np.ascontiguousarray(beta[sl].reshape(1, FC)),
        })
    return in_maps


def kernel(**inputs):
    l = np.asarray(inputs["query"]).shape[1]
    if "nc" not in _cached or _cached.get("l") != l:
        _cached["nc"] = build_program(l)
        _cached["l"] = l
    nc = _cached["nc"]

    in_maps = make_in_maps(inputs, l)
    run_np = get_runner(nc)[0]
    results = run_np(in_maps)

    n = np.asarray(inputs["query"]).shape[0]
    out = np.zeros((n, l, D), dtype=np.float32)
    for c in range(NCORES):
        sl = slice(c * FC, (c + 1) * FC)
        out[:, :, sl] = results[c]["out_s"].reshape(n, l, FC).astype(
            np.float32)
    return out


# revision 60
# speedup vs baseline: 1.3269x; 1.0333x over previous
"""Trainium2 Bass kernel for MultiHeadAttention + residual + BatchNorm.

Model (reference):
  q = query @ Wq.T ; k = key @ Wk.T ; v = key @ Wv.T    (per-head split)
  score = q k^T / sqrt(D), causal mask, softmax over keys
  res   = (attn @ v) + query
  out   = batchnorm(res over all (N*L) rows, per feature) * gamma + beta

Sharding over 8 cores: FEATURE sharding. Core c owns heads {2c, 2c+1}
(features [128c, 128c+128)) for ALL batches. BatchNorm statistics are
then core-local (sums over all N*L rows of the core's own features), so
no collective is needed at all.

All matmul operands are bf16 (PSUM accumulation stays fp32); the
residual add, batch-norm statistics and outputs are fp32.

Per 512-row query chunk ic and batch n the attention inner loop walks
key blocks jc (128 keys each, causal): PE computes both heads' scores
into one PSUM tile [128 j, 1024 (h,i)], ScalarE applies a single merged
exp -> bf16, DVE masks the diagonal block with a triangular multiply,
and PE accumulates the *flipped* attention-V product out[i, p] with a
ones-column appended to V so softmax denominators fall out of the same
matmuls. Projections for chunk ic+1 are emitted as small PE micro-ops
paced between attention iterations so the scalar engine never starves.
"""

import math
import sys

sys.path.insert(0, "/opt/trn_rl_repo")

import numpy as np
import ml_dtypes

import concourse.bass as bass
import concourse.mybir as mybir
from concourse import bacc
import concourse.tile as tile

F32 = mybir.dt.float32
F32R = mybir.dt.float32r
BF16 = mybir.dt.bfloat16
FP8 = mybir.dt.float8e4
BF16_NP = ml_dtypes.bfloat16
FP8_NP = mybir.dt.np(FP8)
# q/k weights are scaled by 16 on the host so fp8e4 stays out of the
# subnormal range; q.k scores come out 256x large, compensated in the
# exp's scale argument
W8_SCALE = 16.0

N = 4
L = 2048
D = 1024
H = 16
P = 64
NCORES = 8
FC = D // NCORES       # features per core = 128
H2 = 2                 # heads per core
EPS = 1e-5
SCALE = 1.0 / math.sqrt(D)
NL = N * L             # 8192 rows in the global batch norm

# software-pipeline stage lags (consumers trail producers so engine wait
# queues never backpressure the sequencers)
EXP_LAG = 1
MASK_LAG = 2
AV_LAG = 3
DEBUG_NOLAG = False

_cached = {}


def r(ap):
    return ap.bitcast(F32R)


def build_program(l=L):
    """Build the SPMD Bass program (identical on all 8 cores)."""
    nc = bacc.Bacc("TRN2", target_bir_lowering=False, debug=False,
                   num_devices=NCORES)

    ic_n = l // 512        # 512-row query chunks
    nlc = N * l            # rows per core (all batches)

    xq8_nd = nc.dram_tensor("xq8_nd", [D, nlc], FP8,
                            kind="ExternalInput").ap()
    xk8_nd = nc.dram_tensor("xk8_nd", [D, nlc], FP8,
                            kind="ExternalInput").ap()
    wq8 = nc.dram_tensor("wq8", [D, FC], FP8, kind="ExternalInput").ap()
    wk8 = nc.dram_tensor("wk8", [D, FC], FP8, kind="ExternalInput").ap()
    wvt = nc.dram_tensor("wvt", [D, FC], FP8, kind="ExternalInput").ap()
    q_res = nc.dram_tensor("q_res", [nlc, FC], BF16, kind="ExternalInput").ap()
    gamma = nc.dram_tensor("gamma", [1, FC], F32, kind="ExternalInput").ap()
    beta = nc.dram_tensor("beta", [1, FC], F32, kind="ExternalInput").ap()
    out_s = nc.dram_tensor("out_s", [nlc, FC], BF16,
                           kind="ExternalOutput").ap()

    jblocks = l // 128     # 16 key blocks per batch
    nls = nlc // 128       # 64 ls blocks of res

    from contextlib import ExitStack
    with ExitStack() as stack:
        tc = stack.enter_context(tile.TileContext(nc))
        pool = {}
        for nm, bufs, space in (
                ("consts", 1, None), ("persist", 1, None), ("wt", 1, None),
                ("qtp", 2, None), ("xq", 2, None), ("xk", 2, None),
                ("xk8", 2, None),
                ("at2", 4, None), ("qin", 2, None), ("sq", 2, None),
                ("outp", 3, None), ("bnp", 1, None), ("small", 6, None),
                ("st2", 2, "PSUM"), ("av", 1, "PSUM"), ("pj", 1, "PSUM"),
                ("stat", 1, "PSUM")):
            kw = {"name": nm, "bufs": bufs}
            if space:
                kw["space"] = space
            pool[nm] = stack.enter_context(tc.tile_pool(**kw))
        consts, persist, wtp = pool["consts"], pool["persist"], pool["wt"]
        qtp, xqp, xkp = pool["qtp"], pool["xq"], pool["xk"]
        xk8p = pool["xk8"]
        at2p, qinp, sqp = pool["at2"], pool["qin"], pool["sq"]
        outp, bnp, smallp = pool["outp"], pool["bnp"], pool["small"]
        st2p, avp, pjp, statp = (pool["st2"], pool["av"], pool["pj"],
                                 pool["stat"])

        # ---------------- first activation chunk DMAs (critical path) ----
        def x_src(dram, n, ic):
            return bass.AP(
                tensor=dram.tensor,
                offset=dram.offset + n * l + ic * 512,
                ap=[[nlc, 128], [128 * nlc, 8], [1, 512]])

        def load_x(pool, dram, n, ic):
            t = pool.tile([128, 8 * 512], BF16, tag="x", name="xt")
            nc.sync.dma_start(
                t.rearrange("p (dc x) -> p dc x", dc=8), x_src(dram, n, ic))
            return t

        def x8_src(dram, n, ic):
            # d = s*256 + t*128 + p (fp8 DoubleRow pair layout)
            return bass.AP(
                tensor=dram.tensor,
                offset=dram.offset + n * l + ic * 512,
                ap=[[nlc, 128], [256 * nlc, 4], [128 * nlc, 2], [1, 512]])

        def load_x8(pool, dram, n, ic):
            t = pool.tile([128, 4 * 2 * 512], FP8, tag="x8", name="x8t")
            nc.sync.dma_start(
                t.rearrange("p (s t x) -> p s t x", s=4, t=2),
                x8_src(dram, n, ic))
            return t

        # first q-projection chain needs wq then xq: emit those two DMAs
        # first so PE can start as early as possible
        wts = {}

        def load_w8(wname, wdram):
            t = wtp.tile([128, 4 * 2 * FC], FP8, tag=wname, name=wname)
            nc.sync.dma_start(
                t.rearrange("p (s t f) -> p s t f", s=4, t=2),
                bass.AP(tensor=wdram.tensor, offset=wdram.offset,
                        ap=[[FC, 128], [256 * FC, 4], [128 * FC, 2],
                            [1, FC]]))
            wts[wname] = t

        load_w8("wq", wq8)
        xq_t = load_x8(xqp, xq8_nd, 0, 0)
        load_w8("wk", wk8)
        xk8_t = load_x8(xk8p, xk8_nd, 0, 0)
        load_w8("wv", wvt)

        # ---------------- constants -------------------------------------
        ones_col = consts.tile([128, 1], BF16)
        nc.vector.memset(ones_col, 1.0)
        eps_sb = consts.tile([128, 1], F32)
        nc.vector.memset(eps_sb, EPS)
        gamma_sb = consts.tile([1, FC], F32)
        nc.sync.dma_start(gamma_sb, gamma)
        beta_sb = consts.tile([1, FC], F32)
        nc.sync.dma_start(beta_sb, beta)
        # lower-triangular (j <= i) mask in [j-part, i-free] layout
        tm_f = consts.tile([128, 128], F32)
        nc.vector.memset(tm_f, 1.0)
        nc.gpsimd.affine_select(
            out=tm_f, in_=tm_f,
            compare_op=mybir.AluOpType.is_ge, fill=0.0, base=0,
            pattern=[[1, 128]], channel_multiplier=-1)
        trimask = consts.tile([128, 128], BF16)
        nc.vector.tensor_copy(trimask, tm_f)
        # zero operands for the PSUM-bank-clearing matmuls (PSUM
        # start_tensor_calc marks a whole 2KB zero-region, so banks shared
        # by several accumulation groups must be cleared by one explicit
        # full-tile start matmul instead of per-group start flags)
        zlhs = consts.tile([128, 128], BF16)
        nc.vector.memset(zlhs, 0.0)
        zrhs = consts.tile([128, 260], BF16)
        nc.vector.memset(zrhs, 0.0)

        # ---------------- persistent SBUF -------------------------------
        # kt_sb: [feature(h*64+p), (n, j)] bf16
        kt_sb = persist.tile([128, N * l], BF16, tag="kt")
        # v_sb: [j-in-block, (n, jc, h, 65)] bf16; col 64 of each 65-group
        # is the baked ones column (softmax denominator trick)
        v_sb = persist.tile([128, N * jblocks * H2 * 65], BF16, tag="v")
        v3 = v_sb.rearrange("p (g x) -> p g x", x=65)
        nc.gpsimd.memset(v3[:, :, 64:65], 1.0)
        # res_sb: [l-in-block, (ls, f)] bf16, ls = n*16 + ic*4 + S
        res_sb = persist.tile([128, nls * FC], BF16, tag="res")

        # partition 0 row: sums; partition 64 row: sums of squares
        # (matmul outputs must start at partition 0, 32, or 64)
        stat = statp.tile([65, 512], F32, tag="stat")

        # ------------------------------------------------------------------
        # projection task machinery (filler micro-ops paced into B loops)
        # ------------------------------------------------------------------
        # rotating PSUM allocators: the interleaved fillers use the single
        # pj bank; the up-front A(0) block also rotates through the two
        # (then idle) score banks so chains overlap their copy-out
        def alloc_pj():
            return pjp.tile([128, 512], F32, tag="pj", name="pj")

        def alloc_st2_slot():
            return st2p.tile([128, 1024], F32, tag="st2", name="st2")[:, 0:512]

        rot = {"i": 0}

        def alloc_rotating():
            rot["i"] += 1
            return alloc_pj() if rot["i"] % 3 == 0 else alloc_st2_slot()

        chain_alloc = {"fn": alloc_pj}

        def qk_chain(side, n, ic, qt_tile, get_xt):
            """q/k projection via fp8 DoubleRow: 4 contraction-256 steps."""
            w_use = wts["wq"] if side == "q" else wts["wk"]
            w4 = w_use.rearrange("p (s t f) -> p s t f", s=4, t=2)
            pj = {}
            alloc_fn = chain_alloc["fn"]

            def alloc():
                pj["t"] = alloc_fn()

            def mm(s0):
                x4 = get_xt().rearrange("p (s t x) -> p s t x", s=4, t=2)
                for s in (s0, s0 + 1):
                    nc.tensor.matmul(
                        pj["t"], w4[:, s], x4[:, s],
                        start=(s == 0), stop=(s == 3),
                        perf_mode=mybir.MatmulPerfMode.DoubleRow)

            def copy():
                if side == "q":
                    nc.vector.tensor_copy(qt_tile[:, n * 512:(n + 1) * 512],
                                          pj["t"])
                else:
                    nc.vector.tensor_copy(
                        kt_sb[:, n * l + ic * 512:n * l + ic * 512 + 512],
                        pj["t"])

            ops = [alloc]
            for s0 in range(0, 4, 2):
                ops.append(lambda s=s0: mm(s))
            ops.append(copy)
            return ops

        def wts_slice(w, dc):
            return w[:, dc * FC:(dc + 1) * FC]

        def v_chain(n, jsub, ic, get_xt):
            pj = {}
            alloc_fn = chain_alloc["fn"]
            wv4 = wts["wv"].rearrange("p (s t f) -> p s t f", s=4, t=2)

            def alloc():
                pj["t"] = alloc_fn()

            def mm(s0):
                x4 = get_xt().rearrange("p (s t x) -> p s t x", s=4, t=2)
                for s in (s0, s0 + 1):
                    nc.tensor.matmul(
                        pj["t"][:, 0:128],
                        x4[:, s, :, jsub * 128:jsub * 128 + 128],
                        wv4[:, s],
                        start=(s == 0), stop=(s == 3),
                        perf_mode=mybir.MatmulPerfMode.DoubleRow)

            def copy():
                jc = ic * 4 + jsub
                base = (n * jblocks + jc) * H2 * 65
                dst = v_sb[:, base:base + 130].rearrange(
                    "p (h x) -> p h x", h=2)[:, :, 0:64]
                src = pj["t"][:, 0:128].rearrange("p (h x) -> p h x", h=2)
                nc.vector.tensor_copy(dst, src)

            ops = [alloc]
            for s0 in range(0, 4, 2):
                ops.append(lambda s=s0: mm(s))
            ops.append(copy)
            return ops

        def build_chunk_groups(ic, qt_tile, first_x, rotate_first=False):
            """Return per-batch lists of micro-op closures for A(ic)."""
            groups = []
            xq_cur = {0: first_x[0]}
            xk8_cur = {0: first_x[1]}
            for n in range(N):
                chain_alloc["fn"] = (alloc_rotating if rotate_first and n == 0
                                     else alloc_pj)
                ops = []
                get_xq = lambda nn=n: xq_cur[nn]
                get_xk8 = lambda nn=n: xk8_cur[nn]
                # prefetch next batch's activations
                if n + 1 < N:
                    def pre(nn=n + 1):
                        xq_cur[nn] = load_x8(xqp, xq8_nd, nn, ic)
                        xk8_cur[nn] = load_x8(xk8p, xk8_nd, nn, ic)
                    ops.append(pre)
                ops += qk_chain("q", n, ic, qt_tile, get_xq)
                ops += qk_chain("k", n, ic, None, get_xk8)
                for jsub in range(4):
                    ops += v_chain(n, jsub, ic, get_xk8)
                groups.append(ops)
            return groups

        # ------------------------------------------------------------------
        # A(0): only batch 0's projections run up front; batches 1-3 are
        # deadline-paced into B(0)'s iterations (batch n is needed at
        # B(0) iteration 4n) so the scalar engine starts exp'ing early.
        # ------------------------------------------------------------------
        qt_next = qtp.tile([128, N * 512], BF16, tag="qt", name="qt")
        groups0 = build_chunk_groups(0, qt_next, (xq_t, xk8_t),
                                     rotate_first=True)
        for op in groups0[0]:
            op()

        # ------------------------------------------------------------------
        # main loop: one software-pipelined stream over (ic, n, jc).
        # Stage schedule at step t: scores(t), exp(t-1), mask(t-2), AV(t-3)
        # so every instruction's inputs are ready when the engine decodes
        # it (the 4-deep per-engine wait queues otherwise backpressure the
        # sequencers). A(ic+1) projection micro-ops are paced in as filler.
        # ------------------------------------------------------------------
        specs = []
        for ic in range(ic_n):
            for n in range(N):
                for jc in range(4 * ic + 4):
                    specs.append((ic, n, jc))
        nspec = len(specs)
        qt_tiles = {0: qt_next}
        st2_of, at2_of, avs_of, qres_of = {}, {}, {}, {}
        filler = []
        quota = {}

        def emit_filler(k):
            for _ in range(k):
                if filler:
                    filler.pop(0)()

        def stage_scores(idx):
            ic, n, jc = specs[idx]
            st2 = st2p.tile([128, 1024], F32, tag="st2", name="st2")
            st2_of[idx] = st2
            qt_cur = qt_tiles[ic]
            for h in range(H2):
                nc.tensor.matmul(
                    st2[:, h * 512:(h + 1) * 512],
                    kt_sb[h * 64:(h + 1) * 64,
                          n * l + jc * 128:n * l + jc * 128 + 128],
                    qt_cur[h * 64:(h + 1) * 64, n * 512:(n + 1) * 512],
                    start=True, stop=True)

        def stage_exp(idx):
            ic, n, jc = specs[idx]
            rr = jc - 4 * ic
            st2 = st2_of.pop(idx)
            at2 = at2p.tile([128, 1024], BF16, tag="at2", name="at2")
            at2_of[idx] = at2
            if rr <= 0:
                nc.scalar.activation(at2, st2,
                                     mybir.ActivationFunctionType.Exp,
                                     scale=SCALE / (W8_SCALE * W8_SCALE))
            else:
                # one strided call covering both heads' causal windows
                a3 = at2.rearrange("p (h x) -> p h x", h=2)[:, :, rr * 128:]
                s3 = st2.rearrange("p (h x) -> p h x", h=2)[:, :, rr * 128:]
                nc.scalar.activation(
                    a3, s3, mybir.ActivationFunctionType.Exp,
                    scale=SCALE / (W8_SCALE * W8_SCALE))

        def stage_mask(idx):
            ic, n, jc = specs[idx]
            rr = jc - 4 * ic
            if rr < 0:
                return
            at2 = at2_of[idx]
            for h in range(H2):
                sl = slice(h * 512 + rr * 128, h * 512 + rr * 128 + 128)
                nc.vector.tensor_mul(at2[:, sl], at2[:, sl], trimask)

        def stage_av(idx):
            ic, n, jc = specs[idx]
            rr = jc - 4 * ic
            at2 = at2_of.pop(idx)
            if jc == 0:
                avs_of[(ic, n)] = [avp.tile([128, 260], F32, tag=f"av{h}",
                                            name=f"av{h}")
                                   for h in range(H2)]
                for h in range(H2):
                    # clear the whole accumulator bank exactly once
                    nc.tensor.matmul(avs_of[(ic, n)][h], zlhs, zrhs,
                                     start=True, stop=True,
                                     skip_group_check=True)
            avs = avs_of[(ic, n)]
            vbase = (n * jblocks + jc) * H2 * 65
            for h in range(H2):
                for S in range(4):
                    if rr > S:
                        continue
                    nc.tensor.matmul(
                        avs[h][:, S * 65:(S + 1) * 65],
                        at2[:, h * 512 + S * 128:h * 512 + S * 128 + 128],
                        v_sb[:, vbase + h * 65:vbase + h * 65 + 65],
                        start=False, stop=(rr == S),
                        skip_group_check=True)
            if jc == 4 * ic + 3:
                enqueue_drain(ic, n)

        # drains and stats run as small deferred pieces, one per iteration,
        # so their engine dependencies are satisfied before dispatch and
        # they never block the in-order PE/DVE queues.
        drain_pending = []

        def enqueue_drain(ic, n):
            avs = avs_of.pop((ic, n))
            qres_t = qres_of.pop((ic, n))
            base512 = (n * 16 + ic * 4) * FC

            def drain_head(h):
                av3 = avs[h].rearrange("p (s x) -> p s x", x=65)
                rec = smallp.tile([128, 4], F32, tag="rec", name="rec")
                nc.vector.reciprocal(rec, av3[:, :, 64])
                for S in range(4):
                    nc.vector.scalar_tensor_tensor(
                        out=res_sb[:, base512 + S * FC + h * 64:
                                   base512 + S * FC + h * 64 + 64],
                        in0=avs[h][:, S * 65:S * 65 + 64],
                        scalar=rec[:, S:S + 1],
                        in1=qres_t[:, S * FC + h * 64:S * FC + h * 64 + 64],
                        op0=mybir.AluOpType.mult,
                        op1=mybir.AluOpType.add)

            def drain_sq():
                res_block = res_sb[:, base512:base512 + 512]
                sqt = sqp.tile([128, 512], BF16, tag="sq", name="sqt")
                nc.vector.tensor_mul(sqt, res_block, res_block)
                stats_bufs[(ic, n)] = (res_block, sqt)

            def drain_stats():
                res_block, sqt = stats_bufs.pop((ic, n))
                first = (n == 0 and ic == 0)
                last = (n == N - 1 and ic == ic_n - 1)
                nc.tensor.matmul(stat[0:1, :], ones_col, res_block,
                                 start=first, stop=last,
                                 skip_group_check=True)
                nc.tensor.matmul(stat[64:65, :], ones_col, sqt,
                                 start=first, stop=last,
                                 skip_group_check=True)

            if DEBUG_NOLAG:
                drain_head(0)
                drain_head(1)
                drain_sq()
                drain_stats()
            else:
                drain_pending.extend(
                    [lambda: drain_head(0), None,
                     lambda: (drain_head(1), drain_sq()), None, drain_stats])

        stats_bufs = {}

        def step_drain(flush=False):
            while drain_pending:
                op = drain_pending.pop(0)
                if op is None:
                    if flush:
                        continue
                    return
                op()

        # chunk 0's remaining projection groups (batches 1-3) are due just
        # before B(0) reaches that batch: group n spread over iters
        # [4(n-1), 4n)
        for n in range(1, N):
            ops = groups0[n]
            no = len(ops)
            for t in range(4):
                quota[4 * (n - 1) + t] = (quota.get(4 * (n - 1) + t, 0)
                                          + ((t + 1) * no) // 4
                                          - (t * no) // 4)
        filler = groups0[1] + groups0[2] + groups0[3]

        for idx in range(nspec + max(EXP_LAG, MASK_LAG, AV_LAG)):
            if idx < nspec:
                ic, n, jc = specs[idx]
                if jc == 0 and n == 0 and ic + 1 < ic_n:
                    # build next chunk's projection fillers, paced over
                    # this chunk's iterations (merged with any deadline
                    # quotas already scheduled for these slots)
                    if ic > 0:
                        emit_filler(len(filler))
                    qt_tiles[ic + 1] = qtp.tile([128, N * 512], BF16,
                                                tag="qt", name="qt")
                    nxq = load_x8(xqp, xq8_nd, 0, ic + 1)
                    nxk8 = load_x8(xk8p, xk8_nd, 0, ic + 1)
                    groups = build_chunk_groups(ic + 1, qt_tiles[ic + 1],
                                                (nxq, nxk8))
                    new_ops = [op for g in groups for op in g]
                    iters = N * (4 * ic + 4)
                    nf = len(new_ops)
                    for t in range(iters):
                        quota[idx + t] = (quota.get(idx + t, 0)
                                          + ((t + 1) * nf) // iters
                                          - (t * nf) // iters)
                    filler.extend(new_ops)
                if jc == 0:
                    qres_t = qinp.tile([128, 4 * FC], BF16, tag="qres",
                                       name="qres")
                    nc.sync.dma_start(
                        qres_t.rearrange("p (s f) -> p s f", s=4),
                        bass.AP(tensor=q_res.tensor,
                                offset=q_res.offset + (n * l + ic * 512) * FC,
                                ap=[[FC, 128], [128 * FC, 4], [1, FC]]))
                    qres_of[(ic, n)] = qres_t
                stage_scores(idx)
            if idx - EXP_LAG >= 0 and idx - EXP_LAG < nspec:
                stage_exp(idx - EXP_LAG)
            if idx - MASK_LAG >= 0 and idx - MASK_LAG < nspec:
                stage_mask(idx - MASK_LAG)
            step_drain()
            if idx - AV_LAG >= 0 and idx - AV_LAG < nspec:
                stage_av(idx - AV_LAG)
            emit_filler(quota.get(idx, 0))
            if idx == nspec - 1:
                emit_filler(len(filler))
        step_drain(flush=True)

        # ------------------------------------------------------------------
        # batch-norm: fold partial sums, compute gamma', beta', apply
        # ------------------------------------------------------------------
        # fold the 4 ls-group partials to [1, FC] on partition 0
        sum_r = bnp.tile([1, 512], F32, tag="sumr", name="sumr")
        nc.vector.tensor_copy(sum_r, stat[0:1, :])
        sq_r = bnp.tile([1, 512], F32, tag="sqr", name="sqr")
        nc.vector.tensor_copy(sq_r, stat[64:65, :])
        sA = bnp.tile([1, FC], F32, tag="sA", name="sA")
        nc.vector.tensor_add(sA, sum_r[:, 0:FC], sum_r[:, FC:2 * FC])
        sB = bnp.tile([1, FC], F32, tag="sB", name="sB")
        nc.vector.tensor_add(sB, sum_r[:, 2 * FC:3 * FC],
                             sum_r[:, 3 * FC:4 * FC])
        sumf = bnp.tile([1, FC], F32, tag="sumf", name="sumf")
        nc.vector.tensor_add(sumf, sA, sB)
        qA = bnp.tile([1, FC], F32, tag="qA", name="qA")
        nc.vector.tensor_add(qA, sq_r[:, 0:FC], sq_r[:, FC:2 * FC])
        qB = bnp.tile([1, FC], F32, tag="qB", name="qB")
        nc.vector.tensor_add(qB, sq_r[:, 2 * FC:3 * FC],
                             sq_r[:, 3 * FC:4 * FC])
        sqf = bnp.tile([1, FC], F32, tag="sqf", name="sqf")
        nc.vector.tensor_add(sqf, qA, qB)

        inv = 1.0 / NL
        mean = bnp.tile([1, FC], F32, tag="mean", name="mean")
        nc.vector.tensor_scalar_mul(mean, sumf, inv)
        musq = bnp.tile([1, FC], F32, tag="musq", name="musq")   # mean^2
        nc.vector.tensor_mul(musq, mean, mean)
        var = bnp.tile([1, FC], F32, tag="var", name="var")
        nc.vector.scalar_tensor_tensor(
            out=var, in0=sqf, scalar=inv, in1=musq,
            op0=mybir.AluOpType.mult, op1=mybir.AluOpType.subtract)
        std = bnp.tile([1, FC], F32, tag="std", name="std")
        nc.scalar.activation(std, var, mybir.ActivationFunctionType.Sqrt,
                             bias=eps_sb[0:1, :])
        rstd = bnp.tile([1, FC], F32, tag="rstd", name="rstd")
        nc.vector.reciprocal(rstd, std)
        gp = bnp.tile([1, FC], F32, tag="gp", name="gp")
        nc.vector.tensor_mul(gp, gamma_sb, rstd)
        mgp = bnp.tile([1, FC], F32, tag="mgp", name="mgp")
        nc.vector.tensor_mul(mgp, mean, gp)
        bp = bnp.tile([1, FC], F32, tag="bp", name="bp")
        nc.vector.tensor_sub(bp, beta_sb, mgp)
        gp16 = bnp.tile([1, FC], BF16, tag="gp16", name="gp16")
        nc.vector.tensor_copy(gp16, gp)
        bp16 = bnp.tile([1, FC], BF16, tag="bp16", name="bp16")
        nc.vector.tensor_copy(bp16, bp)

        gbc = bnp.tile([128, FC], BF16, tag="gbc", name="gbc")
        nc.gpsimd.partition_broadcast(gbc, gp16)
        bbc = bnp.tile([128, FC], BF16, tag="bbc", name="bbc")
        nc.gpsimd.partition_broadcast(bbc, bp16)

        def rep4(t):
            return bass.AP(tensor=t.tensor, offset=t.offset,
                           ap=[[t.ap[0][0], 128], [0, 4], [1, FC]])

        gbc4 = bnp.tile([128, 512], BF16, tag="gbc4", name="gbc4")
        nc.vector.tensor_copy(gbc4, rep4(gbc))
        bbc4 = bnp.tile([128, 512], BF16, tag="bbc4", name="bbc4")
        nc.vector.tensor_copy(bbc4, rep4(bbc))

        for n in range(N):
            for ic in range(ic_n):
                base512 = (n * 16 + ic * 4) * FC
                t1 = outp.tile([128, 512], BF16, tag="t1", name="t1")
                nc.vector.tensor_mul(t1, res_sb[:, base512:base512 + 512],
                                     gbc4)
                t2 = outp.tile([128, 512], BF16, tag="t2", name="t2")
                nc.vector.tensor_add(t2, t1, bbc4)
                nc.sync.dma_start(
                    bass.AP(tensor=out_s.tensor,
                            offset=out_s.offset + (n * l + ic * 512) * FC,
                            ap=[[FC, 128], [128 * FC, 4], [1, FC]]),
                    t2.rearrange("p (s f) -> p s f", s=4))

    nc.compile()
    return nc


def get_runner(nc):
    """Build (once) a cached jitted SPMD executor for the Bass program."""
    if "runner" in _cached:
        return _cached["runner"]

    import jax
    from jax.experimental.shard_map import shard_map
    from jax.sharding import Mesh, PartitionSpec
    from concourse import bass2jax

    bass2jax.install_neuronx_cc_hook()

    partition_name = (nc.partition_id_tensor.name
                      if nc.partition_id_tensor else None)
    in_names, out_names, out_avals, zero_outs = [], [], [], []
    for alloc in nc.m.functions[0].allocations:
        if not isinstance(alloc, mybir.MemoryLocationSet):
            continue
        name = alloc.memorylocations[0].name
        if alloc.kind == "ExternalInput":
            if name != partition_name:
                in_names.append(name)
        elif alloc.kind == "ExternalOutput":
            shape = tuple(alloc.tensor_shape)
            dtype = mybir.dt.np(alloc.dtype)
            out_names.append(name)
            out_avals.append(jax.core.ShapedArray(shape, dtype))
            zero_outs.append(np.zeros(shape, dtype))
    n_params = len(in_names)
    n_outs = len(out_avals)
    all_names = in_names + out_names
    if partition_name is not None:
        all_names = all_names + [partition_name]

    def _body(*args):
        operands = list(args)
        if partition_name is not None:
            operands.append(bass2jax.partition_id_tensor())
        outs = bass2jax._bass_exec_p.bind(
            *operands,
            out_avals=tuple(out_avals),
            in_names=tuple(all_names),
            out_names=tuple(out_names),
            lowering_input_output_aliases=(),
            sim_require_finite=True,
            sim_require_nnan=True,
            nc=nc,
        )
        return tuple(outs)

    devices = jax.devices()[:NCORES]
    mesh = Mesh(np.asarray(devices), ("core",))
    in_specs = (PartitionSpec("core"),) * (n_params + n_outs)
    out_specs = (PartitionSpec("core"),) * n_outs
    donate = tuple(range(n_params, n_params + n_outs))
    sharded = jax.jit(
        shard_map(_body, mesh=mesh, in_specs=in_specs, out_specs=out_specs,
                  check_rep=False),
        donate_argnums=donate, keep_unused=True)

    def run_np(in_maps):
        concat_in = [
            np.concatenate([np.asarray(in_maps[c][nm]) for c in range(NCORES)],
                           axis=0)
            for nm in in_names]
        concat_zeros = [np.zeros((NCORES * z.shape[0], *z.shape[1:]), z.dtype)
                        for z in zero_outs]
        out_arrs = sharded(*concat_in, *concat_zeros)
        return [
            {nm: np.asarray(out_arrs[i]).reshape(
                NCORES, *out_avals[i].shape)[c]
             for i, nm in enumerate(out_names)}
            for c in range(NCORES)]

    _cached["runner"] = (run_np, sharded, in_names, out_names, out_avals,
                         zero_outs, mesh)
    return _cached["runner"]


def make_in_maps(inputs, l):
    query = np.asarray(inputs["query"], dtype=np.float32)
    key = np.asarray(inputs["key"], dtype=np.float32)
    Wq = np.asarray(inputs["Wq"], dtype=np.float32)
    Wk = np.asarray(inputs["Wk"], dtype=np.float32)
    Wv = np.asarray(inputs["Wv"], dtype=np.float32)
    gamma = np.asarray(inputs["gamma"], dtype=np.float32)
    beta = np.asarray(inputs["beta"], dtype=np.float32)

    n = query.shape[0]
    qf = query.reshape(n * l, D)
    kf = key.reshape(n * l, D)
    xq8 = np.ascontiguousarray(qf.T.astype(FP8_NP))
    xk8 = np.ascontiguousarray(kf.T.astype(FP8_NP))

    in_maps = []
    for c in range(NCORES):
        sl = slice(c * FC, (c + 1) * FC)
        in_maps.append({
            "xq8_nd": xq8,
            "xk8_nd": xk8,
            "wq8": np.ascontiguousarray(
                (Wq[sl].T * W8_SCALE).astype(FP8_NP)),
            "wk8": np.ascontiguousarray(
                (Wk[sl].T * W8_SCALE).astype(FP8_NP)),
            "wvt": np.ascontiguousarray(Wv[sl].T.astype(FP8_NP)),
            "q_res": np.ascontiguousarray(qf[:, sl].astype(BF16_NP)),
            "gamma": np.ascontiguousarray(gamma[sl].reshape(1, FC)),
            "beta": np.ascontiguousarray(beta[sl].reshape(1, FC)),
        })
    return in_maps


def kernel(**inputs):
    l = np.asarray(inputs["query"]).shape[1]
    if "nc" not in _cached or _cached.get("l") != l:
        _cached["nc"] = build_program(l)
        _cached["l"] = l
    nc = _cached["nc"]

    in_maps = make_in_maps(inputs, l)
    run_np = get_runner(nc)[0]
    results = run_np(in_maps)

    n = np.asarray(inputs["query"]).shape[0]
    out = np.zeros((n, l, D), dtype=np.float32)
    for c in range(NCORES):
        sl = slice(c * FC, (c + 1) * FC)
        out[:, :, sl] = results[c]["out_s"].reshape(n, l, FC).astype(
            np.float32)
    return out
